# revision 20
# baseline (speedup 1.0000x reference)
"""Trainium2 Bass kernel for nn_DenTargetTransformerConv (GNN message passing).

Strategy (graph/data parallel, dst-owner sharding across 8 NeuronCores):
  - Nodes are partitioned by dst-id range; each core owns N/8 nodes and all
    edges whose dst falls in its range. The "halo exchange" of src features is
    materialized host-side as per-core edge-slot tables (rows replicated per
    consumer), so the device reads are plain strided DMAs.
  - Per core, own nodes are sorted by in-degree and packed into groups of 128
    (SBUF partition dim). Consecutive groups are merged into equal-K runs
    (K = slots per node, shared across the 8 cores so one program serves all).
  - Edge phase per run: one static DMA brings the [128, RK*128] bf16 q||v
    slot tile; DVE computes per-slot q*k products and exp-weighted v in bf16
    (2x mode); the segment reductions (score over D, aggregation over K) run
    on the Tensor engine as identity-weight PSUM-accumulate matmul chains
    (with one DVE pairwise pre-add stage in front to halve the chain length
    where that balances the engines), and the softmax pad-mask is folded in
    as one extra accumulated matmul of a -400 bias table. exp runs on the
    Scalar engine straight out of PSUM and writes its results interleaved
    into the weighted-v tile, so the softmax denominators fall out of the
    aggregation matmuls as 4 extra PSUM columns per group.
  - v (and everything downstream of the aggregation) lives in a (d,h)
    interleaved layout so the exp broadcast has a step-1 inner axis (DVE 2x
    mode); the host un-permutes the final output.
  - The gate's skip-side logit is a linear function of feat, so it is folded
    into the per-node linears as one extra matmul column. The node phase
    (softmax normalize, gate, LayerNorm, PReLU) runs in three group-chunks
    interleaved with the edge runs; broadcast multiplies go to GpSimd,
    transcendentals to the Scalar engine.
  - Emission is software-pipelined: run i's score phase is emitted before
    run i-1's weighted-aggregation phase, so the in-order DVE stream never
    stalls waiting for a PE/ACT round trip.
"""

import numpy as np
import ml_dtypes

import concourse.bacc as bacc
import concourse.bass as bass
import concourse.tile as tile
from concourse import mybir
from concourse.bass_utils import run_bass_kernel_spmd

F32 = mybir.dt.float32
BF16 = mybir.dt.bfloat16
AX = mybir.AxisListType
ALU = mybir.AluOpType
ACTF = mybir.ActivationFunctionType
BF = ml_dtypes.bfloat16

P = 128
NCORES = 8
HD = 64          # H * D
H, D = 4, 16
IN_F = 64
W68 = HD + H     # weighted-v row + denominator columns
KS = 2 * HD + 2  # per-group ks row: k(64) | skip(64) | lgt | pad

RMAX = 7         # max groups per run (agg PSUM: R*68 <= 476)
RKMAX = 96       # max slot-columns per run
KSPREAD = 2     # max K padding when merging groups into a run
NCHUNK = 5       # node-phase chunks
AGGPRE_R = 5     # agg pairwise pre-add for runs with R <= this

# natural hd = h*16+d  <->  stored j = d*4+h
_PERM = np.arange(HD).reshape(H, D).T.reshape(-1)       # j -> natural hd


# ----------------------------------------------------------------- host prep

def _plan(q_src, v_src, feat, src, dst, ncores):
    n = feat.shape[0]
    npc = n // ncores
    ngrp = (npc + P - 1) // P
    grid = ngrp * P
    ndum = grid - npc

    q2 = np.asarray(q_src, np.float32).reshape(n, HD)
    v2 = np.asarray(v_src, np.float32).reshape(n, H, D).transpose(0, 2, 1).reshape(n, HD)
    qv = np.concatenate([q2, v2], axis=1).astype(BF)    # [n, 128]

    src = np.asarray(src).astype(np.int64)
    dst = np.asarray(dst).astype(np.int64)
    order = np.argsort(dst, kind="stable")
    dst_s, src_s = dst[order], src[order]
    bounds = np.searchsorted(dst_s, np.arange(ncores + 1) * npc)

    cores = []
    gmax = np.zeros((ncores, ngrp), np.int64)
    for c in range(ncores):
        lo, hi = bounds[c], bounds[c + 1]
        dstL = dst_s[lo:hi] - c * npc          # ascending
        srcL = src_s[lo:hi]
        deg = np.bincount(dstL, minlength=npc)
        starts = np.concatenate([[0], np.cumsum(deg)])
        rank = np.arange(len(dstL)) - starts[dstL]
        perm = np.argsort(deg, kind="stable")  # ascending degree
        pos_of = np.empty(npc, np.int64)
        pos_of[perm] = ndum + np.arange(npc)
        gd = np.zeros(grid, np.int64)
        gd[ndum:] = deg[perm]
        gmax[c] = gd.reshape(ngrp, P).max(1)
        cores.append(dict(dstL=dstL, srcL=srcL, rank=rank, perm=perm,
                          pos_of=pos_of))

    K = np.maximum(gmax.max(0), 1)             # per-group slot count

    # merge consecutive groups into equal-K runs (pad K up to the run max)
    runs = []
    g = 0
    while g < ngrp:
        ge = g + 1
        while (ge < ngrp and ge - g < RMAX
               and (ge - g + 1) * K[ge] <= RKMAX
               and K[ge] - K[g] <= KSPREAD):
            ge += 1
        runs.append((g, ge, int(K[ge - 1])))
        g = ge
    rkbase = np.zeros(len(runs) + 1, np.int64)
    for i, (g0, g1, k) in enumerate(runs):
        rkbase[i + 1] = rkbase[i] + (g1 - g0) * k
    totrk = int(rkbase[-1])

    # per-core tables
    per_core = []
    grp_run = np.zeros(ngrp, np.int64)
    for i, (g0, g1, k) in enumerate(runs):
        grp_run[g0:g1] = i
    run_g0 = np.array([r[0] for r in runs])
    run_k = np.array([r[2] for r in runs])

    for c in range(ncores):
        cd = cores[c]
        pos_e = cd["pos_of"][cd["dstL"]]       # grid position of each edge
        g_e = pos_e // P
        p_e = pos_e % P
        i_e = grp_run[g_e]
        r_e = g_e - run_g0[i_e]
        k_e = run_k[i_e]
        # row = rkbase[i]*128 + p*(R*K) + r*K + rank  (partition-major)
        rk_run = np.array([r[1] - r[0] for r in runs])[i_e] * k_e
        rows = rkbase[i_e] * P + p_e * rk_run + r_e * k_e + cd["rank"]
        tab = np.zeros((totrk * P, 2 * HD), BF)
        tab[rows] = qv[cd["srcL"]]
        # padded slots have q=v=0 -> score 0 -> exp 1; count them per node
        # so the denominator can be corrected (eps folded in)
        nslot = np.zeros(ngrp, np.int64)
        for i, (g0, g1, k) in enumerate(runs):
            nslot[g0:g1] = k
        real = np.zeros((P, ngrp), np.float32)
        np.add.at(real, (p_e, g_e), 1.0)
        padc = nslot[None, :] - real - 1e-9
        padcnt = np.repeat(padc, H, axis=1).astype(np.float32)  # [128, G*4]
        per_core.append(dict(tab=tab, padcnt=padcnt))

    # featT with ones row, per core, grid-permuted: [IN_F+1, grid] bf16
    featTs = []
    feat = np.asarray(feat, np.float32)
    for c in range(ncores):
        ft = np.zeros((IN_F + 1, grid), np.float32)
        ft[IN_F, :] = 1.0
        perm = cores[c]["perm"]
        ft[:IN_F, ndum:] = feat[c * npc + perm].T
        featTs.append(ft.astype(BF))

    ident = np.eye(P, dtype=BF)

    return dict(n=n, npc=npc, ngrp=ngrp, grid=grid, ndum=ndum, K=K,
                runs=runs, rkbase=rkbase, totrk=totrk, ident=ident,
                cores=cores, per_core=per_core, featTs=featTs)


# ------------------------------------------------------------- device build

def _build_nc(plan, ncores):
    ngrp, runs, rkbase, totrk = (plan["ngrp"], plan["runs"], plan["rkbase"],
                                 plan["totrk"])
    grid = plan["grid"]
    G = ngrp
    nruns = len(runs)

    nc = bacc.Bacc("TRN2", target_bir_lowering=False, debug=False,
                   num_devices=ncores)

    featT_d = nc.dram_tensor("featT", [IN_F + 1, grid], BF16,
                             kind="ExternalInput").ap()
    FT_G = [12, 12, 12, G - 36]  # featT split sizes (groups)
    tab_d = nc.dram_tensor("tab", [totrk * P, 2 * HD], BF16,
                           kind="ExternalInput").ap()
    padc_d = nc.dram_tensor("padcnt", [P, G * H], F32,
                            kind="ExternalInput").ap()
    ident_d = nc.dram_tensor("ident", [P, P], BF16, kind="ExternalInput").ap()
    wcat_d = nc.dram_tensor("wcat", [IN_F + 1, KS], BF16,
                            kind="ExternalInput").ap()
    # bf16 params: [wg2' | gamma' | beta'] (all (d,h)-permuted)
    parb_d = nc.dram_tensor("parb", [1, 3 * HD], BF16,
                            kind="ExternalInput").ap()
    # f32 params: [bgate, prelu_a, eps, pad]
    parf_d = nc.dram_tensor("parf", [1, 4], F32, kind="ExternalInput").ap()
    out_d = nc.dram_tensor("out", [P, G * HD], F32, kind="ExternalOutput").ap()

    # node-phase chunk boundaries; chunk j is emitted in the pipeline slot
    # right after run chunk_done[j]'s aggregation phase
    cw = [12, 12, 11, 9, 5]          # chunk sizes, small tail
    assert sum(cw) == G and len(cw) == NCHUNK
    cb = [sum(cw[:i]) for i in range(NCHUNK + 1)]
    chunk_done = {}
    for j in range(NCHUNK):
        i = min(i for i, (g0, g1, k) in enumerate(runs) if g1 >= cb[j + 1])
        chunk_done.setdefault(i, []).append((cb[j], cb[j + 1]))
    NGMAX = max(cb[j + 1] - cb[j] for j in range(NCHUNK))

    with tile.TileContext(nc) as tc:
        with (
            tc.tile_pool(name="singles", bufs=1) as singles,
            tc.tile_pool(name="plin", bufs=2, space="PSUM") as plin,
            tc.tile_pool(name="pscore", bufs=2, space="PSUM") as pscore,
            tc.tile_pool(name="pagg", bufs=2, space="PSUM") as pagg,
            tc.tile_pool(name="qvp", bufs=3) as qvp,
            tc.tile_pool(name="prodp", bufs=2) as prodp,
            tc.tile_pool(name="halfp", bufs=2) as halfp,
            tc.tile_pool(name="wp", bufs=2) as wp,
            tc.tile_pool(name="whp", bufs=2) as whp,
            tc.tile_pool(name="nodep", bufs=2) as nodep,
            tc.tile_pool(name="smallp", bufs=2) as smallp,
        ):
            qv_pre = {}

            def qv_fetch(i):
                g0, g1, K = runs[i]
                RK = (g1 - g0) * K
                r0 = int(rkbase[i])
                qv = qvp.tile([P, RKMAX * 2 * HD], BF16, tag="qv")
                in_ap = tab_d[r0 * P:(r0 + RK) * P, :].rearrange(
                    "(p rk) e -> p (rk e)", p=P)
                nc.sync.dma_start(out=qv[:, :RK * 2 * HD], in_=in_ap)
                qv_pre[i] = qv

            # ---- prefetch the first run's edge table before anything
            qv_fetch(0)

            # ---- static loads (featT split so linears start early)
            featTs = []
            fg0 = 0
            for ng in FT_G:
                t = singles.tile([IN_F + 1, ng * P], BF16, tag=f"ft{fg0}")
                nc.sync.dma_start(
                    out=t[:], in_=featT_d[:, fg0 * P:(fg0 + ng) * P])
                featTs.append((fg0, fg0 + ng, t))
                fg0 += ng

            def feat_slice(g):
                for a, b, t in featTs:
                    if a <= g < b:
                        return t[:, (g - a) * P:(g - a + 1) * P]
                raise AssertionError
            wcat = singles.tile([IN_F + 1, KS], BF16)
            nc.sync.dma_start(out=wcat[:], in_=wcat_d[:])
            ident = singles.tile([P, P], BF16)
            nc.sync.dma_start(out=ident[:], in_=ident_d[:])
            padcnt = singles.tile([P, G * H], F32)
            nc.sync.dma_start(out=padcnt[:], in_=padc_d[:])
            parb = singles.tile([P, 3 * HD], BF16)
            nc.sync.dma_start(
                out=parb[:],
                in_=bass.AP(tensor=parb_d.tensor, offset=parb_d.offset,
                            ap=[[0, P], [1, 3 * HD]]))
            parf = singles.tile([P, 4], F32)
            nc.sync.dma_start(
                out=parf[:],
                in_=bass.AP(tensor=parf_d.tensor, offset=parf_d.offset,
                            ap=[[0, P], [1, 4]]))
            bg_ap = parf[:, 0:1]
            nbg_ap = parf[:, 3:4]
            pa_ap = parf[:, 1:2]
            eps_ap = parf[:, 2:3]

            # persistent state
            ks_bf = singles.tile([P, G * KS], BF16)   # k | skip | lgt | pad
            den = singles.tile([P, G * H], F32)
            agg_bf = singles.tile([P, G * HD], BF16)

            def pap(t, extra, off=0):
                sl = t[:, 0:1]
                return bass.AP(tensor=sl.tensor, offset=sl.offset + off,
                               ap=[sl.ap[0]] + extra)

            # ---- per-node linears: k|skip|lgt = featT_g.T @ wcat
            for c0 in range(0, G, 3):
                cn = min(3, G - c0)
                pl = plin.tile([P, 3 * KS], F32, tag="lin")
                for j in range(cn):
                    nc.tensor.matmul(out=pl[:, j * KS:(j + 1) * KS],
                                     lhsT=feat_slice(c0 + j),
                                     rhs=wcat[:], start=True, stop=True)
                nc.scalar.activation(out=ks_bf[:, c0 * KS:(c0 + cn) * KS],
                                     in_=pl[:, :cn * KS], func=ACTF.Copy)

            # ---- edge phase (software-pipelined emission)
            state = {}

            def score_phase(i):
                g0, g1, K = runs[i]
                R = g1 - g0
                RK = R * K
                r0 = int(rkbase[i])
                if i not in qv_pre:
                    qv_fetch(i)
                qv = qv_pre.pop(i)

                # prod[rk, h, d] = q[rk, h, d] * k_g[h, d]  (bf16 2x)
                prod = prodp.tile([P, RKMAX * HD], BF16, tag="prod")
                q3 = pap(qv, [[2 * HD * K, R], [2 * HD, K], [1, HD]])
                kb = pap(ks_bf, [[KS, R], [0, K], [1, HD]], off=g0 * KS)
                p3 = pap(prod, [[HD * K, R], [HD, K], [1, HD]])
                nc.vector.tensor_tensor(out=p3, in0=q3, in1=kb, op=ALU.mult)

                # one pairwise pre-add: 16 d-slices -> 8
                ph = halfp.tile([P, RKMAX * 32], BF16, tag="ph")
                nc.vector.tensor_tensor(
                    out=pap(ph, [[32, RK], [8, H], [1, 8]]),
                    in0=pap(prod, [[HD, RK], [D, H], [1, 8]]),
                    in1=pap(prod, [[HD, RK], [D, H], [1, 8]], off=8),
                    op=ALU.add)

                # score[rk, h] = sum_d8 ph  (PE accumulate)
                sp = pscore.tile([P, RKMAX * H], F32, tag="sp")
                for d in range(8):
                    rhs = pap(ph, [[32, RK], [8, H]], off=d)
                    nc.tensor.matmul(out=sp[:, :RK * H], lhsT=ident[:],
                                     rhs=rhs, start=(d == 0), stop=(d == 7))

                # w68 cols 64:68 = ex = exp(score/4)  (ACT, PSUM -> SBUF)
                w = wp.tile([P, RKMAX * W68], BF16, tag="w")
                nc.scalar.activation(
                    out=pap(w, [[W68, RK], [1, H]], off=HD),
                    in_=sp[:, :RK * H], func=ACTF.Exp, scale=0.25)
                state[i] = (qv, w)

            def agg_phase(i):
                g0, g1, K = runs[i]
                R = g1 - g0
                RK = R * K
                qv, w = state.pop(i)

                # w68[rk, 0:64] = v[rk, d, h] * ex[rk, h]  (bf16 2x)
                v3 = pap(qv, [[2 * HD, RK], [H, D], [1, H]], off=HD)
                eb = pap(w, [[W68, RK], [0, D], [1, H]], off=HD)
                w3 = pap(w, [[W68, RK], [H, D], [1, H]])
                nc.vector.tensor_tensor(out=w3, in0=v3, in1=eb, op=ALU.mult)

                # optional pairwise k pre-add for narrow runs
                ag = pagg.tile([P, RMAX * W68], F32, tag="agg")
                if R <= AGGPRE_R and K > 1:
                    KH = K // 2
                    wh = whp.tile([P, (RKMAX // 2 + 1) * W68], BF16, tag="wh")
                    nc.vector.tensor_tensor(
                        out=pap(wh, [[KH * W68, R], [W68, KH], [1, W68]]),
                        in0=pap(w, [[K * W68, R], [2 * W68, KH], [1, W68]]),
                        in1=pap(w, [[K * W68, R], [2 * W68, KH], [1, W68]],
                                off=W68),
                        op=ALU.add)
                    nmm = KH + (K % 2)
                    for k in range(KH):
                        rhs = pap(wh, [[KH * W68, R], [1, W68]], off=k * W68)
                        nc.tensor.matmul(out=ag[:, :R * W68], lhsT=ident[:],
                                         rhs=rhs, start=(k == 0),
                                         stop=(k == nmm - 1))
                    if K % 2:
                        rhs = pap(w, [[K * W68, R], [1, W68]],
                                  off=(K - 1) * W68)
                        nc.tensor.matmul(out=ag[:, :R * W68], lhsT=ident[:],
                                         rhs=rhs, start=(KH == 0), stop=True)
                else:
                    for k in range(K):
                        rhs = pap(w, [[K * W68, R], [1, W68]], off=k * W68)
                        nc.tensor.matmul(out=ag[:, :R * W68], lhsT=ident[:],
                                         rhs=rhs, start=(k == 0),
                                         stop=(k == K - 1))

                nc.scalar.activation(
                    out=pap(agg_bf, [[HD, R], [1, HD]], off=g0 * HD),
                    in_=pap(ag, [[W68, R], [1, HD]]),
                    func=ACTF.Copy)
                nc.scalar.activation(
                    out=pap(den, [[H, R], [1, H]], off=g0 * H),
                    in_=pap(ag, [[W68, R], [1, H]], off=HD),
                    func=ACTF.Copy)

            nstate = {}

            def node_chunk_a(h0, h1):
                NG = h1 - h0
                Fh = NG * HD
                bce = nc.vector if h1 == G else nc.gpsimd

                dv = smallp.tile([P, NGMAX * H], F32, tag="dinv")
                nc.vector.tensor_tensor(out=dv[:, :NG * H],
                                        in0=den[:, h0 * H:h1 * H],
                                        in1=padcnt[:, h0 * H:h1 * H],
                                        op=ALU.subtract)
                nc.vector.reciprocal_approx_fast(out=dv[:, :NG * H],
                                                 in_=dv[:, :NG * H])
                dvb = smallp.tile([P, NGMAX * H], BF16, tag="dinvb")
                nc.vector.tensor_copy(out=dvb[:, :NG * H],
                                      in_=dv[:, :NG * H])

                # rst = agg * dinv
                rst = nodep.tile([P, NGMAX * HD], BF16, tag="rst")
                dib = pap(dvb, [[H, NG], [0, D], [1, H]])
                a3 = pap(agg_bf, [[HD, NG], [H, D], [1, H]], off=h0 * HD)
                r3 = pap(rst, [[HD, NG], [H, D], [1, H]])
                nc.vector.tensor_tensor(out=r3, in0=a3, in1=dib, op=ALU.mult)

                # gate logit: lgt_skip (from linears) + sum_hd rst*wg2
                zc = nodep.tile([P, NGMAX * HD], BF16, tag="zc")
                sk = pap(ks_bf, [[KS, NG], [1, HD]], off=h0 * KS + HD)
                wg2 = pap(parb, [[0, NG], [1, HD]], off=0)
                nc.vector.tensor_tensor(out=zc[:, :Fh], in0=rst[:, :Fh],
                                        in1=wg2, op=ALU.mult)
                lgs = smallp.tile([P, NGMAX], F32, tag="lgs")
                nc.vector.tensor_reduce(
                    out=lgs[:, :NG],
                    in_=pap(zc, [[HD, NG], [1, HD]]),
                    axis=AX.X, op=ALU.add)
                logit = smallp.tile([P, NGMAX], F32, tag="logit")
                lgtb = pap(ks_bf, [[KS, NG], [1, 1]], off=h0 * KS + 2 * HD)
                nc.vector.tensor_tensor(out=logit[:, :NG], in0=lgs[:, :NG],
                                        in1=lgtb, op=ALU.add)
                ge = smallp.tile([P, NGMAX], F32, tag="ge")
                nc.scalar.activation(out=ge[:, :NG], in_=logit[:, :NG],
                                     func=ACTF.Exp, scale=-1.0, bias=nbg_ap)
                nc.vector.tensor_scalar(out=ge[:, :NG], in0=ge[:, :NG],
                                        scalar1=1.0, scalar2=None,
                                        op0=ALU.add)
                nc.vector.reciprocal_approx_fast(out=ge[:, :NG],
                                                 in_=ge[:, :NG])
                gate = smallp.tile([P, NGMAX], BF16, tag="gate")
                nc.vector.tensor_copy(out=gate[:, :NG], in_=ge[:, :NG])

                # rst += gate * (skip - rst)
                dif = nodep.tile([P, NGMAX * HD], BF16, tag="dif")
                nc.vector.tensor_tensor(out=dif[:, :Fh], in0=sk,
                                        in1=rst[:, :Fh], op=ALU.subtract)
                gb = pap(gate, [[1, NG], [0, HD]])
                d3 = pap(dif, [[HD, NG], [1, HD]])
                bce.tensor_tensor(out=d3, in0=d3, in1=gb, op=ALU.mult)
                nstate[h0] = (h1, rst, dif, bce)

            def node_chunk_bc(h0):
                h1, rst, dif, bce = nstate.pop(h0)
                NG = h1 - h0
                Fh = NG * HD
                nc.vector.tensor_tensor(out=rst[:, :Fh], in0=rst[:, :Fh],
                                        in1=dif[:, :Fh], op=ALU.add)

                # LayerNorm stats: sum rst (DVE), sum rst^2 (ACT square)
                sq = nodep.tile([P, NGMAX * HD], BF16, tag="zc2")
                nc.scalar.activation(out=sq[:, :Fh], in_=rst[:, :Fh],
                                     func=ACTF.Square)
                stats = smallp.tile([P, 2 * NGMAX], F32, tag="stats")
                nc.vector.tensor_reduce(
                    out=stats[:, :NG],
                    in_=pap(rst, [[HD, NG], [1, HD]]),
                    axis=AX.X, op=ALU.add)
                nc.vector.tensor_reduce(
                    out=stats[:, NG:2 * NG],
                    in_=pap(sq, [[HD, NG], [1, HD]]),
                    axis=AX.X, op=ALU.add)
                nc.vector.tensor_scalar(out=stats[:, :2 * NG],
                                        in0=stats[:, :2 * NG],
                                        scalar1=1.0 / HD, scalar2=None,
                                        op0=ALU.mult)
                mu = stats[:, 0:NG]
                msq = stats[:, NG:2 * NG]
                var = smallp.tile([P, NGMAX], F32, tag="var")
                nc.vector.tensor_tensor(out=var[:, :NG], in0=mu, in1=mu,
                                        op=ALU.mult)
                nc.vector.tensor_tensor(out=var[:, :NG], in0=msq,
                                        in1=var[:, :NG], op=ALU.subtract)
                sd = smallp.tile([P, NGMAX], F32, tag="sd")
                nc.scalar.activation(out=sd[:, :NG], in_=var[:, :NG],
                                     func=ACTF.Sqrt, bias=eps_ap)
                nc.vector.reciprocal_approx_fast(out=sd[:, :NG],
                                                 in_=sd[:, :NG])
                mrs = smallp.tile([P, 2 * NGMAX], BF16, tag="mrs")
                nc.vector.tensor_copy(out=mrs[:, :NG], in_=mu)
                nc.vector.tensor_copy(out=mrs[:, NG:2 * NG], in_=sd[:, :NG])

                # xhat = (rst - mu) * rstd; out = prelu(xhat*gamma + beta)
                mub = pap(mrs, [[1, NG], [0, HD]])
                bce.tensor_tensor(out=rst[:, :Fh], in0=rst[:, :Fh],
                                  in1=mub, op=ALU.subtract)
                rsb = pap(mrs, [[1, NG], [0, HD]], off=NG)
                bce.tensor_tensor(out=rst[:, :Fh], in0=rst[:, :Fh],
                                  in1=rsb, op=ALU.mult)
                gmb = pap(parb, [[0, NG], [1, HD]], off=HD)
                nc.vector.tensor_tensor(out=rst[:, :Fh], in0=rst[:, :Fh],
                                        in1=gmb, op=ALU.mult)
                btb = pap(parb, [[0, NG], [1, HD]], off=2 * HD)
                nc.vector.tensor_tensor(out=rst[:, :Fh], in0=rst[:, :Fh],
                                        in1=btb, op=ALU.add)
                outf = nodep.tile([P, NGMAX * HD], F32, tag="outf")
                nc.scalar.activation(out=outf[:, :Fh], in_=rst[:, :Fh],
                                     func=ACTF.Prelu, alpha=pa_ap)
                nc.sync.dma_start(out=out_d[:, h0 * HD:h1 * HD],
                                  in_=outf[:, :Fh])

            pend = []
            for i in range(nruns + 1):
                if i < nruns:
                    score_phase(i)
                for h0 in pend:
                    node_chunk_bc(h0)
                pend = []
                if i > 0:
                    agg_phase(i - 1)
                    for h0, h1 in chunk_done.get(i - 1, []):
                        node_chunk_a(h0, h1)
                        pend.append(h0)
            for h0 in pend:
                node_chunk_bc(h0)

    nc.compile()
    return nc


# ------------------------------------------------------------------- driver

_CACHE = {}


def _get_nc(plan, ncores):
    key = (tuple(int(k) for g0, g1, k in plan["runs"]),
           tuple(g1 - g0 for g0, g1, k in plan["runs"]),
           plan["grid"], ncores)
    if key not in _CACHE:
        _CACHE[key] = _build_nc(plan, ncores)
    return _CACHE[key]


def _make_inmaps(plan, params, ncores):
    (Wk, bk, Wskip, bskip, Wgate, bgate, ln_gamma, ln_beta, prelu_a) = params
    Wk = np.asarray(Wk, np.float32)
    bk = np.asarray(bk, np.float32)
    Wsk = np.asarray(Wskip, np.float32)
    bsk = np.asarray(bskip, np.float32)
    wg = np.asarray(Wgate, np.float32).reshape(3 * HD)
    wg1n = wg[0:HD] + wg[2 * HD:3 * HD]                 # acts on skip (nat)
    wcat = np.zeros((IN_F + 1, KS), np.float32)
    wcat[:IN_F, :HD] = Wk
    wcat[IN_F, :HD] = bk
    wcat[:IN_F, HD:2 * HD] = Wsk[:, _PERM]
    wcat[IN_F, HD:2 * HD] = bsk[_PERM]
    wcat[:IN_F, 2 * HD] = Wsk @ wg1n                    # skip-side gate logit
    wcat[IN_F, 2 * HD] = bsk @ wg1n
    wcat = wcat.astype(BF)

    parb = np.zeros((1, 3 * HD), np.float32)
    parb[0, 0:HD] = (wg[HD:2 * HD] - wg[2 * HD:3 * HD])[_PERM]   # on rst
    parb[0, HD:2 * HD] = np.asarray(ln_gamma, np.float32)[_PERM]
    parb[0, 2 * HD:3 * HD] = np.asarray(ln_beta, np.float32)[_PERM]
    parb = parb.astype(BF)
    parf = np.zeros((1, 4), np.float32)
    parf[0, 0] = np.float32(np.asarray(bgate).reshape(-1)[0])
    parf[0, 1] = np.float32(np.asarray(prelu_a).reshape(-1)[0])
    parf[0, 2] = 1e-5
    parf[0, 3] = -parf[0, 0]

    in_maps = []
    for c in range(ncores):
        pc = plan["per_core"][c]
        m = dict(featT=plan["featTs"][c], tab=pc["tab"],
                 padcnt=pc["padcnt"], ident=plan["ident"],
                 wcat=wcat, parb=parb, parf=parf)
        in_maps.append(m)
    return in_maps


def run(q_src, v_src, feat, src, dst, Wk, bk, Wskip, bskip, Wgate, bgate,
        ln_gamma, ln_beta, prelu_a, ncores=NCORES, trace=False):
    plan = _plan(q_src, v_src, feat, src, dst, ncores)
    nc = _get_nc(plan, ncores)
    in_maps = _make_inmaps(
        plan, (Wk, bk, Wskip, bskip, Wgate, bgate, ln_gamma, ln_beta, prelu_a),
        ncores)
    res = run_bass_kernel_spmd(nc, in_maps, core_ids=list(range(ncores)),
                               trace=trace)
    n, npc, ngrp = plan["n"], plan["npc"], plan["ngrp"]
    out = np.empty((n, HD), np.float32)
    for c in range(ncores):
        r = np.asarray(res.results[c]["out"])              # [128, ngrp*64]
        r = r.reshape(P, ngrp, D, H).transpose(1, 0, 3, 2)  # -> [g, p, h, d]
        arr = r.reshape(-1, HD)
        out[c * npc + plan["cores"][c]["perm"]] = \
            arr[plan["ndum"]:plan["ndum"] + npc]
    return out, res, plan, in_maps, nc


def kernel(**inputs):
    out, _, _, _, _ = run(**inputs)
    return out


# revision 21
# speedup vs baseline: 1.0278x; 1.0278x over previous
"""Trainium2 Bass kernel for nn_DenTargetTransformerConv (GNN message passing).

Strategy (graph/data parallel, dst-owner sharding across 8 NeuronCores):
  - Nodes are partitioned by dst-id range; each core owns N/8 nodes and all
    edges whose dst falls in its range. The "halo exchange" of src features is
    materialized host-side as per-core edge-slot tables (rows replicated per
    consumer), so the device reads are plain strided DMAs.
  - Per core, own nodes are sorted by in-degree and packed into groups of 128
    (SBUF partition dim). Consecutive groups are merged into equal-K runs
    (K = slots per node, shared across the 8 cores so one program serves all).
  - Edge phase per run: one static DMA brings the [128, RK*128] bf16 q||v
    slot tile; DVE computes per-slot q*k products and exp-weighted v in bf16
    (2x mode); the segment reductions (score over D, aggregation over K) run
    on the Tensor engine as identity-weight PSUM-accumulate matmul chains
    (with one DVE pairwise pre-add stage in front to halve the chain length
    where that balances the engines), and the softmax pad-mask is folded in
    as one extra accumulated matmul of a -400 bias table. exp runs on the
    Scalar engine straight out of PSUM and writes its results interleaved
    into the weighted-v tile, so the softmax denominators fall out of the
    aggregation matmuls as 4 extra PSUM columns per group.
  - v (and everything downstream of the aggregation) lives in a (d,h)
    interleaved layout so the exp broadcast has a step-1 inner axis (DVE 2x
    mode); the host un-permutes the final output.
  - The gate's skip-side logit is a linear function of feat, so it is folded
    into the per-node linears as one extra matmul column. The node phase
    (softmax normalize, gate, LayerNorm, PReLU) runs in three group-chunks
    interleaved with the edge runs; broadcast multiplies go to GpSimd,
    transcendentals to the Scalar engine.
  - Emission is software-pipelined: run i's score phase is emitted before
    run i-1's weighted-aggregation phase, so the in-order DVE stream never
    stalls waiting for a PE/ACT round trip.
"""

import numpy as np
import ml_dtypes

import concourse.bacc as bacc
import concourse.bass as bass
import concourse.tile as tile
from concourse import mybir
from concourse.bass_utils import run_bass_kernel_spmd

F32 = mybir.dt.float32
BF16 = mybir.dt.bfloat16
AX = mybir.AxisListType
ALU = mybir.AluOpType
ACTF = mybir.ActivationFunctionType
BF = ml_dtypes.bfloat16

P = 128
NCORES = 8
HD = 64          # H * D
H, D = 4, 16
IN_F = 64
W68 = HD + H     # weighted-v row + denominator columns
KS = 2 * HD + 2  # per-group ks row: k(64) | skip(64) | lgt | pad

RMAX = 7         # max groups per run (agg PSUM: R*68 <= 476)
RKMAX = 96       # max slot-columns per run
KSPREAD = 2     # max K padding when merging groups into a run
NCHUNK = 5       # node-phase chunks
AGGPRE_R = 5     # agg pairwise pre-add for runs with R <= this

# natural hd = h*16+d  <->  stored j = d*4+h
_PERM = np.arange(HD).reshape(H, D).T.reshape(-1)       # j -> natural hd


# ----------------------------------------------------------------- host prep

def _plan(q_src, v_src, feat, src, dst, ncores):
    n = feat.shape[0]
    npc = n // ncores
    ngrp = (npc + P - 1) // P
    grid = ngrp * P
    ndum = grid - npc

    q2 = np.asarray(q_src, np.float32).reshape(n, HD)
    v2 = np.asarray(v_src, np.float32).reshape(n, H, D).transpose(0, 2, 1).reshape(n, HD)
    qv = np.concatenate([q2, v2], axis=1).astype(BF)    # [n, 128]

    src = np.asarray(src).astype(np.int64)
    dst = np.asarray(dst).astype(np.int64)
    order = np.argsort(dst, kind="stable")
    dst_s, src_s = dst[order], src[order]
    bounds = np.searchsorted(dst_s, np.arange(ncores + 1) * npc)

    cores = []
    gmax = np.zeros((ncores, ngrp), np.int64)
    for c in range(ncores):
        lo, hi = bounds[c], bounds[c + 1]
        dstL = dst_s[lo:hi] - c * npc          # ascending
        srcL = src_s[lo:hi]
        deg = np.bincount(dstL, minlength=npc)
        starts = np.concatenate([[0], np.cumsum(deg)])
        rank = np.arange(len(dstL)) - starts[dstL]
        perm = np.argsort(deg, kind="stable")  # ascending degree
        pos_of = np.empty(npc, np.int64)
        pos_of[perm] = ndum + np.arange(npc)
        gd = np.zeros(grid, np.int64)
        gd[ndum:] = deg[perm]
        gmax[c] = gd.reshape(ngrp, P).max(1)
        cores.append(dict(dstL=dstL, srcL=srcL, rank=rank, perm=perm,
                          pos_of=pos_of))

    K = np.maximum(gmax.max(0), 1)             # per-group slot count

    # merge consecutive groups into equal-K runs (pad K up to the run max)
    runs = []
    g = 0
    while g < ngrp:
        ge = g + 1
        while (ge < ngrp and ge - g < RMAX
               and (ge - g + 1) * K[ge] <= RKMAX
               and K[ge] - K[g] <= KSPREAD):
            ge += 1
        runs.append((g, ge, int(K[ge - 1])))
        g = ge
    rkbase = np.zeros(len(runs) + 1, np.int64)
    for i, (g0, g1, k) in enumerate(runs):
        rkbase[i + 1] = rkbase[i] + (g1 - g0) * k
    totrk = int(rkbase[-1])

    # per-core tables
    per_core = []
    grp_run = np.zeros(ngrp, np.int64)
    for i, (g0, g1, k) in enumerate(runs):
        grp_run[g0:g1] = i
    run_g0 = np.array([r[0] for r in runs])
    run_k = np.array([r[2] for r in runs])

    for c in range(ncores):
        cd = cores[c]
        pos_e = cd["pos_of"][cd["dstL"]]       # grid position of each edge
        g_e = pos_e // P
        p_e = pos_e % P
        i_e = grp_run[g_e]
        r_e = g_e - run_g0[i_e]
        k_e = run_k[i_e]
        # row = rkbase[i]*128 + p*(R*K) + r*K + rank  (partition-major)
        rk_run = np.array([r[1] - r[0] for r in runs])[i_e] * k_e
        rows = rkbase[i_e] * P + p_e * rk_run + r_e * k_e + cd["rank"]
        tab = np.zeros((totrk * P, 2 * HD), BF)
        tab[rows] = qv[cd["srcL"]]
        # padded slots have q=v=0 -> score 0 -> exp 1; count them per node
        # so the denominator can be corrected (eps folded in)
        nslot = np.zeros(ngrp, np.int64)
        for i, (g0, g1, k) in enumerate(runs):
            nslot[g0:g1] = k
        real = np.zeros((P, ngrp), np.float32)
        np.add.at(real, (p_e, g_e), 1.0)
        padc = nslot[None, :] - real - 1e-9
        padcnt = np.repeat(padc, H, axis=1).astype(np.float32)  # [128, G*4]
        per_core.append(dict(tab=tab, padcnt=padcnt))

    # featT with ones row, per core, grid-permuted: [IN_F+1, grid] bf16
    featTs = []
    feat = np.asarray(feat, np.float32)
    for c in range(ncores):
        ft = np.zeros((IN_F + 1, grid), np.float32)
        ft[IN_F, :] = 1.0
        perm = cores[c]["perm"]
        ft[:IN_F, ndum:] = feat[c * npc + perm].T
        featTs.append(ft.astype(BF))

    ident = np.eye(P, dtype=BF)

    return dict(n=n, npc=npc, ngrp=ngrp, grid=grid, ndum=ndum, K=K,
                runs=runs, rkbase=rkbase, totrk=totrk, ident=ident,
                cores=cores, per_core=per_core, featTs=featTs)


# ------------------------------------------------------------- device build

def _build_nc(plan, ncores):
    ngrp, runs, rkbase, totrk = (plan["ngrp"], plan["runs"], plan["rkbase"],
                                 plan["totrk"])
    grid = plan["grid"]
    G = ngrp
    nruns = len(runs)

    nc = bacc.Bacc("TRN2", target_bir_lowering=False, debug=False,
                   num_devices=ncores)

    featT_d = nc.dram_tensor("featT", [IN_F + 1, grid], BF16,
                             kind="ExternalInput").ap()
    FT_G = [12, 12, 12, G - 36]  # featT split sizes (groups)
    tab_d = nc.dram_tensor("tab", [totrk * P, 2 * HD], BF16,
                           kind="ExternalInput").ap()
    padc_d = nc.dram_tensor("padcnt", [P, G * H], F32,
                            kind="ExternalInput").ap()
    ident_d = nc.dram_tensor("ident", [P, P], BF16, kind="ExternalInput").ap()
    wcat_d = nc.dram_tensor("wcat", [IN_F + 1, KS], BF16,
                            kind="ExternalInput").ap()
    # bf16 params: [wg2' | gamma' | beta'] (all (d,h)-permuted)
    parb_d = nc.dram_tensor("parb", [1, 3 * HD], BF16,
                            kind="ExternalInput").ap()
    # f32 params: [bgate, prelu_a, eps, pad]
    parf_d = nc.dram_tensor("parf", [1, 4], F32, kind="ExternalInput").ap()
    out_d = nc.dram_tensor("out", [P, G * HD], F32, kind="ExternalOutput").ap()

    # node-phase chunk boundaries; chunk j is emitted in the pipeline slot
    # right after run chunk_done[j]'s aggregation phase
    cw = [12, 12, 11, 9, 5]          # chunk sizes, small tail
    assert sum(cw) == G and len(cw) == NCHUNK
    cb = [sum(cw[:i]) for i in range(NCHUNK + 1)]
    chunk_done = {}
    for j in range(NCHUNK):
        i = min(i for i, (g0, g1, k) in enumerate(runs) if g1 >= cb[j + 1])
        chunk_done.setdefault(i, []).append((cb[j], cb[j + 1]))
    NGMAX = max(cb[j + 1] - cb[j] for j in range(NCHUNK))

    with tile.TileContext(nc) as tc:
        with (
            tc.tile_pool(name="singles", bufs=1) as singles,
            tc.tile_pool(name="plin", bufs=2, space="PSUM") as plin,
            tc.tile_pool(name="pscore", bufs=2, space="PSUM") as pscore,
            tc.tile_pool(name="pagg", bufs=2, space="PSUM") as pagg,
            tc.tile_pool(name="qvp", bufs=3) as qvp,
            tc.tile_pool(name="prodp", bufs=2) as prodp,
            tc.tile_pool(name="halfp", bufs=2) as halfp,
            tc.tile_pool(name="wp", bufs=2) as wp,
            tc.tile_pool(name="whp", bufs=2) as whp,
            tc.tile_pool(name="nodep", bufs=2) as nodep,
            tc.tile_pool(name="smallp", bufs=2) as smallp,
        ):
            qv_pre = {}

            def qv_fetch(i):
                g0, g1, K = runs[i]
                RK = (g1 - g0) * K
                r0 = int(rkbase[i])
                qv = qvp.tile([P, RKMAX * 2 * HD], BF16, tag="qv")
                in_ap = tab_d[r0 * P:(r0 + RK) * P, :].rearrange(
                    "(p rk) e -> p (rk e)", p=P)
                nc.sync.dma_start(out=qv[:, :RK * 2 * HD], in_=in_ap)
                qv_pre[i] = qv

            # ---- prefetch the first runs' edge tables before anything
            qv_fetch(0)
            qv_fetch(1)

            # ---- static loads (featT split so linears start early)
            featTs = []
            fg0 = 0
            for ng in FT_G:
                t = singles.tile([IN_F + 1, ng * P], BF16, tag=f"ft{fg0}")
                nc.sync.dma_start(
                    out=t[:], in_=featT_d[:, fg0 * P:(fg0 + ng) * P])
                featTs.append((fg0, fg0 + ng, t))
                fg0 += ng

            def feat_slice(g):
                for a, b, t in featTs:
                    if a <= g < b:
                        return t[:, (g - a) * P:(g - a + 1) * P]
                raise AssertionError
            wcat = singles.tile([IN_F + 1, KS], BF16)
            nc.sync.dma_start(out=wcat[:], in_=wcat_d[:])
            ident = singles.tile([P, P], BF16)
            nc.sync.dma_start(out=ident[:], in_=ident_d[:])
            padcnt = singles.tile([P, G * H], F32)
            nc.sync.dma_start(out=padcnt[:], in_=padc_d[:])
            parb = singles.tile([P, 3 * HD], BF16)
            nc.sync.dma_start(
                out=parb[:],
                in_=bass.AP(tensor=parb_d.tensor, offset=parb_d.offset,
                            ap=[[0, P], [1, 3 * HD]]))
            parf = singles.tile([P, 4], F32)
            nc.sync.dma_start(
                out=parf[:],
                in_=bass.AP(tensor=parf_d.tensor, offset=parf_d.offset,
                            ap=[[0, P], [1, 4]]))
            bg_ap = parf[:, 0:1]
            nbg_ap = parf[:, 3:4]
            pa_ap = parf[:, 1:2]
            eps_ap = parf[:, 2:3]

            # persistent state
            ks_bf = singles.tile([P, G * KS], BF16)   # k | skip | lgt | pad
            den = singles.tile([P, G * H], F32)
            agg_bf = singles.tile([P, G * HD], BF16)

            def pap(t, extra, off=0):
                sl = t[:, 0:1]
                return bass.AP(tensor=sl.tensor, offset=sl.offset + off,
                               ap=[sl.ap[0]] + extra)

            # ---- per-node linears: k|skip|lgt = featT_g.T @ wcat
            for c0 in range(0, G, 3):
                cn = min(3, G - c0)
                pl = plin.tile([P, 3 * KS], F32, tag="lin")
                for j in range(cn):
                    nc.tensor.matmul(out=pl[:, j * KS:(j + 1) * KS],
                                     lhsT=feat_slice(c0 + j),
                                     rhs=wcat[:], start=True, stop=True)
                nc.scalar.activation(out=ks_bf[:, c0 * KS:(c0 + cn) * KS],
                                     in_=pl[:, :cn * KS], func=ACTF.Copy)

            # ---- edge phase (software-pipelined emission)
            state = {}

            def score_phase(i):
                g0, g1, K = runs[i]
                R = g1 - g0
                RK = R * K
                r0 = int(rkbase[i])
                if i not in qv_pre:
                    qv_fetch(i)
                qv = qv_pre.pop(i)

                # prod[rk, h, d] = q[rk, h, d] * k_g[h, d]  (bf16 2x)
                prod = prodp.tile([P, RKMAX * HD], BF16, tag="prod")
                q3 = pap(qv, [[2 * HD * K, R], [2 * HD, K], [1, HD]])
                kb = pap(ks_bf, [[KS, R], [0, K], [1, HD]], off=g0 * KS)
                p3 = pap(prod, [[HD * K, R], [HD, K], [1, HD]])
                nc.vector.tensor_tensor(out=p3, in0=q3, in1=kb, op=ALU.mult)

                # two pairwise pre-adds: 16 d-slices -> 4
                ph = halfp.tile([P, RKMAX * 32], BF16, tag="ph")
                nc.vector.tensor_tensor(
                    out=pap(ph, [[32, RK], [8, H], [1, 8]]),
                    in0=pap(prod, [[HD, RK], [D, H], [1, 8]]),
                    in1=pap(prod, [[HD, RK], [D, H], [1, 8]], off=8),
                    op=ALU.add)
                pq = halfp.tile([P, RKMAX * 16], BF16, tag="pq")
                nc.vector.tensor_tensor(
                    out=pap(pq, [[16, RK], [4, H], [1, 4]]),
                    in0=pap(ph, [[32, RK], [8, H], [1, 4]]),
                    in1=pap(ph, [[32, RK], [8, H], [1, 4]], off=4),
                    op=ALU.add)

                # score[rk, h] = sum_d4 pq  (PE accumulate)
                sp = pscore.tile([P, RKMAX * H], F32, tag="sp")
                for d in range(4):
                    rhs = pap(pq, [[16, RK], [4, H]], off=d)
                    nc.tensor.matmul(out=sp[:, :RK * H], lhsT=ident[:],
                                     rhs=rhs, start=(d == 0), stop=(d == 3))

                # w68 cols 64:68 = ex = exp(score/4)  (ACT, PSUM -> SBUF)
                w = wp.tile([P, RKMAX * W68], BF16, tag="w")
                nc.scalar.activation(
                    out=pap(w, [[W68, RK], [1, H]], off=HD),
                    in_=sp[:, :RK * H], func=ACTF.Exp, scale=0.25)
                state[i] = (qv, w)

            def agg_phase(i):
                g0, g1, K = runs[i]
                R = g1 - g0
                RK = R * K
                qv, w = state.pop(i)

                # w68[rk, 0:64] = v[rk, d, h] * ex[rk, h]  (bf16 2x)
                v3 = pap(qv, [[2 * HD, RK], [H, D], [1, H]], off=HD)
                eb = pap(w, [[W68, RK], [0, D], [1, H]], off=HD)
                w3 = pap(w, [[W68, RK], [H, D], [1, H]])
                nc.vector.tensor_tensor(out=w3, in0=v3, in1=eb, op=ALU.mult)

                # optional pairwise k pre-add for narrow runs
                ag = pagg.tile([P, RMAX * W68], F32, tag="agg")
                if R <= AGGPRE_R and K > 1:
                    KH = K // 2
                    wh = whp.tile([P, (RKMAX // 2 + 1) * W68], BF16, tag="wh")
                    nc.vector.tensor_tensor(
                        out=pap(wh, [[KH * W68, R], [W68, KH], [1, W68]]),
                        in0=pap(w, [[K * W68, R], [2 * W68, KH], [1, W68]]),
                        in1=pap(w, [[K * W68, R], [2 * W68, KH], [1, W68]],
                                off=W68),
                        op=ALU.add)
                    nmm = KH + (K % 2)
                    for k in range(KH):
                        rhs = pap(wh, [[KH * W68, R], [1, W68]], off=k * W68)
                        nc.tensor.matmul(out=ag[:, :R * W68], lhsT=ident[:],
                                         rhs=rhs, start=(k == 0),
                                         stop=(k == nmm - 1))
                    if K % 2:
                        rhs = pap(w, [[K * W68, R], [1, W68]],
                                  off=(K - 1) * W68)
                        nc.tensor.matmul(out=ag[:, :R * W68], lhsT=ident[:],
                                         rhs=rhs, start=(KH == 0), stop=True)
                else:
                    for k in range(K):
                        rhs = pap(w, [[K * W68, R], [1, W68]], off=k * W68)
                        nc.tensor.matmul(out=ag[:, :R * W68], lhsT=ident[:],
                                         rhs=rhs, start=(k == 0),
                                         stop=(k == K - 1))

                nc.scalar.activation(
                    out=pap(agg_bf, [[HD, R], [1, HD]], off=g0 * HD),
                    in_=pap(ag, [[W68, R], [1, HD]]),
                    func=ACTF.Copy)
                nc.scalar.activation(
                    out=pap(den, [[H, R], [1, H]], off=g0 * H),
                    in_=pap(ag, [[W68, R], [1, H]], off=HD),
                    func=ACTF.Copy)

            nstate = {}

            def node_chunk_a(h0, h1):
                NG = h1 - h0
                Fh = NG * HD
                bce = nc.vector if h1 == G else nc.gpsimd

                dv = smallp.tile([P, NGMAX * H], F32, tag="dinv")
                nc.vector.tensor_tensor(out=dv[:, :NG * H],
                                        in0=den[:, h0 * H:h1 * H],
                                        in1=padcnt[:, h0 * H:h1 * H],
                                        op=ALU.subtract)
                nc.vector.reciprocal_approx_fast(out=dv[:, :NG * H],
                                                 in_=dv[:, :NG * H])
                dvb = smallp.tile([P, NGMAX * H], BF16, tag="dinvb")
                nc.vector.tensor_copy(out=dvb[:, :NG * H],
                                      in_=dv[:, :NG * H])

                # rst = agg * dinv
                rst = nodep.tile([P, NGMAX * HD], BF16, tag="rst")
                dib = pap(dvb, [[H, NG], [0, D], [1, H]])
                a3 = pap(agg_bf, [[HD, NG], [H, D], [1, H]], off=h0 * HD)
                r3 = pap(rst, [[HD, NG], [H, D], [1, H]])
                nc.vector.tensor_tensor(out=r3, in0=a3, in1=dib, op=ALU.mult)

                # gate logit: lgt_skip (from linears) + sum_hd rst*wg2
                zc = nodep.tile([P, NGMAX * HD], BF16, tag="zc")
                sk = pap(ks_bf, [[KS, NG], [1, HD]], off=h0 * KS + HD)
                wg2 = pap(parb, [[0, NG], [1, HD]], off=0)
                nc.vector.tensor_tensor(out=zc[:, :Fh], in0=rst[:, :Fh],
                                        in1=wg2, op=ALU.mult)
                lgs = smallp.tile([P, NGMAX], F32, tag="lgs")
                nc.vector.tensor_reduce(
                    out=lgs[:, :NG],
                    in_=pap(zc, [[HD, NG], [1, HD]]),
                    axis=AX.X, op=ALU.add)
                logit = smallp.tile([P, NGMAX], F32, tag="logit")
                lgtb = pap(ks_bf, [[KS, NG], [1, 1]], off=h0 * KS + 2 * HD)
                nc.vector.tensor_tensor(out=logit[:, :NG], in0=lgs[:, :NG],
                                        in1=lgtb, op=ALU.add)
                ge = smallp.tile([P, NGMAX], F32, tag="ge")
                nc.scalar.activation(out=ge[:, :NG], in_=logit[:, :NG],
                                     func=ACTF.Exp, scale=-1.0, bias=nbg_ap)
                nc.vector.tensor_scalar(out=ge[:, :NG], in0=ge[:, :NG],
                                        scalar1=1.0, scalar2=None,
                                        op0=ALU.add)
                nc.vector.reciprocal_approx_fast(out=ge[:, :NG],
                                                 in_=ge[:, :NG])
                gate = smallp.tile([P, NGMAX], BF16, tag="gate")
                nc.vector.tensor_copy(out=gate[:, :NG], in_=ge[:, :NG])

                # rst += gate * (skip - rst)
                dif = nodep.tile([P, NGMAX * HD], BF16, tag="dif")
                nc.vector.tensor_tensor(out=dif[:, :Fh], in0=sk,
                                        in1=rst[:, :Fh], op=ALU.subtract)
                gb = pap(gate, [[1, NG], [0, HD]])
                d3 = pap(dif, [[HD, NG], [1, HD]])
                bce.tensor_tensor(out=d3, in0=d3, in1=gb, op=ALU.mult)
                nstate[h0] = (h1, rst, dif, bce)

            def node_chunk_bc(h0):
                h1, rst, dif, bce = nstate.pop(h0)
                NG = h1 - h0
                Fh = NG * HD
                nc.vector.tensor_tensor(out=rst[:, :Fh], in0=rst[:, :Fh],
                                        in1=dif[:, :Fh], op=ALU.add)

                # LayerNorm stats: sum rst (DVE), sum rst^2 (ACT square)
                sq = nodep.tile([P, NGMAX * HD], BF16, tag="zc2")
                nc.scalar.activation(out=sq[:, :Fh], in_=rst[:, :Fh],
                                     func=ACTF.Square)
                stats = smallp.tile([P, 2 * NGMAX], F32, tag="stats")
                nc.vector.tensor_reduce(
                    out=stats[:, :NG],
                    in_=pap(rst, [[HD, NG], [1, HD]]),
                    axis=AX.X, op=ALU.add)
                nc.vector.tensor_reduce(
                    out=stats[:, NG:2 * NG],
                    in_=pap(sq, [[HD, NG], [1, HD]]),
                    axis=AX.X, op=ALU.add)
                nc.vector.tensor_scalar(out=stats[:, :2 * NG],
                                        in0=stats[:, :2 * NG],
                                        scalar1=1.0 / HD, scalar2=None,
                                        op0=ALU.mult)
                mu = stats[:, 0:NG]
                msq = stats[:, NG:2 * NG]
                var = smallp.tile([P, NGMAX], F32, tag="var")
                nc.vector.tensor_tensor(out=var[:, :NG], in0=mu, in1=mu,
                                        op=ALU.mult)
                nc.vector.tensor_tensor(out=var[:, :NG], in0=msq,
                                        in1=var[:, :NG], op=ALU.subtract)
                sd = smallp.tile([P, NGMAX], F32, tag="sd")
                nc.scalar.activation(out=sd[:, :NG], in_=var[:, :NG],
                                     func=ACTF.Sqrt, bias=eps_ap)
                nc.vector.reciprocal_approx_fast(out=sd[:, :NG],
                                                 in_=sd[:, :NG])
                mrs = smallp.tile([P, 2 * NGMAX], BF16, tag="mrs")
                nc.vector.tensor_copy(out=mrs[:, :NG], in_=mu)
                nc.vector.tensor_copy(out=mrs[:, NG:2 * NG], in_=sd[:, :NG])

                # xhat = (rst - mu) * rstd; out = prelu(xhat*gamma + beta)
                mub = pap(mrs, [[1, NG], [0, HD]])
                bce.tensor_tensor(out=rst[:, :Fh], in0=rst[:, :Fh],
                                  in1=mub, op=ALU.subtract)
                rsb = pap(mrs, [[1, NG], [0, HD]], off=NG)
                bce.tensor_tensor(out=rst[:, :Fh], in0=rst[:, :Fh],
                                  in1=rsb, op=ALU.mult)
                gmb = pap(parb, [[0, NG], [1, HD]], off=HD)
                nc.vector.tensor_tensor(out=rst[:, :Fh], in0=rst[:, :Fh],
                                        in1=gmb, op=ALU.mult)
                btb = pap(parb, [[0, NG], [1, HD]], off=2 * HD)
                nc.vector.tensor_tensor(out=rst[:, :Fh], in0=rst[:, :Fh],
                                        in1=btb, op=ALU.add)
                outf = nodep.tile([P, NGMAX * HD], F32, tag="outf")
                nc.scalar.activation(out=outf[:, :Fh], in_=rst[:, :Fh],
                                     func=ACTF.Prelu, alpha=pa_ap)
                nc.sync.dma_start(out=out_d[:, h0 * HD:h1 * HD],
                                  in_=outf[:, :Fh])

            pend = []
            for i in range(nruns + 1):
                if i < nruns:
                    score_phase(i)
                for h0 in pend:
                    node_chunk_bc(h0)
                pend = []
                if i > 0:
                    agg_phase(i - 1)
                    for h0, h1 in chunk_done.get(i - 1, []):
                        node_chunk_a(h0, h1)
                        pend.append(h0)
            for h0 in pend:
                node_chunk_bc(h0)

    nc.compile()
    return nc


# ------------------------------------------------------------------- driver

_CACHE = {}


def _get_nc(plan, ncores):
    key = (tuple(int(k) for g0, g1, k in plan["runs"]),
           tuple(g1 - g0 for g0, g1, k in plan["runs"]),
           plan["grid"], ncores)
    if key not in _CACHE:
        _CACHE[key] = _build_nc(plan, ncores)
    return _CACHE[key]


def _make_inmaps(plan, params, ncores):
    (Wk, bk, Wskip, bskip, Wgate, bgate, ln_gamma, ln_beta, prelu_a) = params
    Wk = np.asarray(Wk, np.float32)
    bk = np.asarray(bk, np.float32)
    Wsk = np.asarray(Wskip, np.float32)
    bsk = np.asarray(bskip, np.float32)
    wg = np.asarray(Wgate, np.float32).reshape(3 * HD)
    wg1n = wg[0:HD] + wg[2 * HD:3 * HD]                 # acts on skip (nat)
    wcat = np.zeros((IN_F + 1, KS), np.float32)
    wcat[:IN_F, :HD] = Wk
    wcat[IN_F, :HD] = bk
    wcat[:IN_F, HD:2 * HD] = Wsk[:, _PERM]
    wcat[IN_F, HD:2 * HD] = bsk[_PERM]
    wcat[:IN_F, 2 * HD] = Wsk @ wg1n                    # skip-side gate logit
    wcat[IN_F, 2 * HD] = bsk @ wg1n
    wcat = wcat.astype(BF)

    parb = np.zeros((1, 3 * HD), np.float32)
    parb[0, 0:HD] = (wg[HD:2 * HD] - wg[2 * HD:3 * HD])[_PERM]   # on rst
    parb[0, HD:2 * HD] = np.asarray(ln_gamma, np.float32)[_PERM]
    parb[0, 2 * HD:3 * HD] = np.asarray(ln_beta, np.float32)[_PERM]
    parb = parb.astype(BF)
    parf = np.zeros((1, 4), np.float32)
    parf[0, 0] = np.float32(np.asarray(bgate).reshape(-1)[0])
    parf[0, 1] = np.float32(np.asarray(prelu_a).reshape(-1)[0])
    parf[0, 2] = 1e-5
    parf[0, 3] = -parf[0, 0]

    in_maps = []
    for c in range(ncores):
        pc = plan["per_core"][c]
        m = dict(featT=plan["featTs"][c], tab=pc["tab"],
                 padcnt=pc["padcnt"], ident=plan["ident"],
                 wcat=wcat, parb=parb, parf=parf)
        in_maps.append(m)
    return in_maps


def run(q_src, v_src, feat, src, dst, Wk, bk, Wskip, bskip, Wgate, bgate,
        ln_gamma, ln_beta, prelu_a, ncores=NCORES, trace=False):
    plan = _plan(q_src, v_src, feat, src, dst, ncores)
    nc = _get_nc(plan, ncores)
    in_maps = _make_inmaps(
        plan, (Wk, bk, Wskip, bskip, Wgate, bgate, ln_gamma, ln_beta, prelu_a),
        ncores)
    res = run_bass_kernel_spmd(nc, in_maps, core_ids=list(range(ncores)),
                               trace=trace)
    n, npc, ngrp = plan["n"], plan["npc"], plan["ngrp"]
    out = np.empty((n, HD), np.float32)
    for c in range(ncores):
        r = np.asarray(res.results[c]["out"])              # [128, ngrp*64]
        r = r.reshape(P, ngrp, D, H).transpose(1, 0, 3, 2)  # -> [g, p, h, d]
        arr = r.reshape(-1, HD)
        out[c * npc + plan["cores"][c]["perm"]] = \
            arr[plan["ndum"]:plan["ndum"] + npc]
    return out, res, plan, in_maps, nc


def kernel(**inputs):
    out, _, _, _, _ = run(**inputs)
    return out


# revision 27
# speedup vs baseline: 1.0931x; 1.0636x over previous
"""Trainium2 Bass kernel for nn_DenTargetTransformerConv (GNN message passing).

Strategy (graph/data parallel, dst-owner sharding across 8 NeuronCores):
  - Nodes are partitioned by dst-id range; each core owns N/8 nodes and all
    edges whose dst falls in its range. The "halo exchange" of src features is
    materialized host-side as per-core edge-slot tables (rows replicated per
    consumer), so the device reads are plain strided DMAs.
  - Per core, own nodes are sorted by in-degree and packed into groups of 128
    (SBUF partition dim). Consecutive groups are merged into equal-K runs
    (K = slots per node, shared across the 8 cores so one program serves all).
  - Edge phase per run: one static DMA brings the [128, RK*128] bf16 q||v
    slot tile; DVE computes per-slot q*k products and exp-weighted v in bf16
    (2x mode); the segment reductions (score over D, aggregation over K) run
    on the Tensor engine as identity-weight PSUM-accumulate matmul chains
    (with one DVE pairwise pre-add stage in front to halve the chain length
    where that balances the engines), and the softmax pad-mask is folded in
    as one extra accumulated matmul of a -400 bias table. exp runs on the
    Scalar engine straight out of PSUM and writes its results interleaved
    into the weighted-v tile, so the softmax denominators fall out of the
    aggregation matmuls as 4 extra PSUM columns per group.
  - v (and everything downstream of the aggregation) lives in a (d,h)
    interleaved layout so the exp broadcast has a step-1 inner axis (DVE 2x
    mode); the host un-permutes the final output.
  - The gate's skip-side logit is a linear function of feat, so it is folded
    into the per-node linears as one extra matmul column. The node phase
    (softmax normalize, gate, LayerNorm, PReLU) runs in three group-chunks
    interleaved with the edge runs; broadcast multiplies go to GpSimd,
    transcendentals to the Scalar engine.
  - Emission is software-pipelined: run i's score phase is emitted before
    run i-1's weighted-aggregation phase, so the in-order DVE stream never
    stalls waiting for a PE/ACT round trip.
"""

import numpy as np
import ml_dtypes

import concourse.bacc as bacc
import concourse.bass as bass
import concourse.tile as tile
from concourse import mybir
from concourse.bass_utils import run_bass_kernel_spmd

F32 = mybir.dt.float32
BF16 = mybir.dt.bfloat16
AX = mybir.AxisListType
ALU = mybir.AluOpType
ACTF = mybir.ActivationFunctionType
BF = ml_dtypes.bfloat16

P = 128
NCORES = 8
HD = 64          # H * D
H, D = 4, 16
IN_F = 64
W68 = HD + H     # weighted-v row + denominator columns
KS = 2 * HD + 2  # per-group ks row: k(64) | skip(64) | lgt | pad

RMAX = 7         # max groups per run (agg PSUM: R*68 <= 476)
RKMAX = 96       # max slot-columns per run
KSPREAD = 2     # max K padding when merging groups into a run
NCHUNK = 5       # node-phase chunks
AGGPRE_R = 4     # agg pairwise pre-add for runs with R <= this

# natural hd = h*16+d  <->  stored j = d*4+h
_PERM = np.arange(HD).reshape(H, D).T.reshape(-1)       # j -> natural hd


# ----------------------------------------------------------------- host prep

def _plan(q_src, v_src, feat, src, dst, ncores):
    n = feat.shape[0]
    npc = n // ncores
    ngrp = (npc + P - 1) // P
    grid = ngrp * P
    ndum = grid - npc

    q2 = np.asarray(q_src, np.float32).reshape(n, HD)
    v2 = np.asarray(v_src, np.float32).reshape(n, H, D).transpose(0, 2, 1).reshape(n, HD)
    qv = np.concatenate([q2, v2], axis=1).astype(BF)    # [n, 128]

    src = np.asarray(src).astype(np.int64)
    dst = np.asarray(dst).astype(np.int64)
    order = np.argsort(dst, kind="stable")
    dst_s, src_s = dst[order], src[order]
    bounds = np.searchsorted(dst_s, np.arange(ncores + 1) * npc)

    cores = []
    gmax = np.zeros((ncores, ngrp), np.int64)
    for c in range(ncores):
        lo, hi = bounds[c], bounds[c + 1]
        dstL = dst_s[lo:hi] - c * npc          # ascending
        srcL = src_s[lo:hi]
        deg = np.bincount(dstL, minlength=npc)
        starts = np.concatenate([[0], np.cumsum(deg)])
        rank = np.arange(len(dstL)) - starts[dstL]
        perm = np.argsort(deg, kind="stable")  # ascending degree
        pos_of = np.empty(npc, np.int64)
        pos_of[perm] = ndum + np.arange(npc)
        gd = np.zeros(grid, np.int64)
        gd[ndum:] = deg[perm]
        gmax[c] = gd.reshape(ngrp, P).max(1)
        cores.append(dict(dstL=dstL, srcL=srcL, rank=rank, perm=perm,
                          pos_of=pos_of))

    K = np.maximum(gmax.max(0), 1)             # per-group slot count

    # merge consecutive groups into equal-K runs (pad K up to the run max)
    runs = []
    g = 0
    while g < ngrp:
        ge = g + 1
        while (ge < ngrp and ge - g < RMAX
               and (ge - g + 1) * K[ge] <= RKMAX
               and K[ge] - K[g] <= KSPREAD):
            ge += 1
        runs.append((g, ge, int(K[ge - 1])))
        g = ge
    rkbase = np.zeros(len(runs) + 1, np.int64)
    for i, (g0, g1, k) in enumerate(runs):
        rkbase[i + 1] = rkbase[i] + (g1 - g0) * k
    totrk = int(rkbase[-1])

    # per-core tables
    per_core = []
    grp_run = np.zeros(ngrp, np.int64)
    for i, (g0, g1, k) in enumerate(runs):
        grp_run[g0:g1] = i
    run_g0 = np.array([r[0] for r in runs])
    run_k = np.array([r[2] for r in runs])

    for c in range(ncores):
        cd = cores[c]
        pos_e = cd["pos_of"][cd["dstL"]]       # grid position of each edge
        g_e = pos_e // P
        p_e = pos_e % P
        i_e = grp_run[g_e]
        r_e = g_e - run_g0[i_e]
        k_e = run_k[i_e]
        # row = rkbase[i]*128 + p*(R*K) + r*K + rank  (partition-major)
        rk_run = np.array([r[1] - r[0] for r in runs])[i_e] * k_e
        rows = rkbase[i_e] * P + p_e * rk_run + r_e * k_e + cd["rank"]
        tabq = np.zeros((totrk * P, HD), BF)
        tabv = np.zeros((totrk * P, HD), BF)
        tabq[rows] = qv[cd["srcL"], :HD]
        tabv[rows] = qv[cd["srcL"], HD:]
        # padded slots have q=v=0 -> score 0 -> exp 1; count them per node
        # so the denominator can be corrected (eps folded in)
        nslot = np.zeros(ngrp, np.int64)
        for i, (g0, g1, k) in enumerate(runs):
            nslot[g0:g1] = k
        real = np.zeros((P, ngrp), np.float32)
        np.add.at(real, (p_e, g_e), 1.0)
        padc = nslot[None, :] - real - 1e-9
        padcnt = np.repeat(padc, H, axis=1).astype(np.float32)  # [128, G*4]
        per_core.append(dict(tabq=tabq, tabv=tabv, padcnt=padcnt))

    # featT with ones row, per core, grid-permuted: [IN_F+1, grid] bf16
    featTs = []
    feat = np.asarray(feat, np.float32)
    for c in range(ncores):
        ft = np.zeros((IN_F + 1, grid), np.float32)
        ft[IN_F, :] = 1.0
        perm = cores[c]["perm"]
        ft[:IN_F, ndum:] = feat[c * npc + perm].T
        featTs.append(ft.astype(BF))

    ident = np.eye(P, dtype=BF)

    return dict(n=n, npc=npc, ngrp=ngrp, grid=grid, ndum=ndum, K=K,
                runs=runs, rkbase=rkbase, totrk=totrk, ident=ident,
                cores=cores, per_core=per_core, featTs=featTs)


# ------------------------------------------------------------- device build

def _build_nc(plan, ncores):
    ngrp, runs, rkbase, totrk = (plan["ngrp"], plan["runs"], plan["rkbase"],
                                 plan["totrk"])
    grid = plan["grid"]
    G = ngrp
    nruns = len(runs)

    nc = bacc.Bacc("TRN2", target_bir_lowering=False, debug=False,
                   num_devices=ncores)

    featT_d = nc.dram_tensor("featT", [IN_F + 1, grid], BF16,
                             kind="ExternalInput").ap()
    FT_G = [12, 12, 12, G - 36]  # featT split sizes (groups)
    tabq_d = nc.dram_tensor("tabq", [totrk * P, HD], BF16,
                            kind="ExternalInput").ap()
    tabv_d = nc.dram_tensor("tabv", [totrk * P, HD], BF16,
                            kind="ExternalInput").ap()
    padc_d = nc.dram_tensor("padcnt", [P, G * H], F32,
                            kind="ExternalInput").ap()
    ident_d = nc.dram_tensor("ident", [P, P], BF16, kind="ExternalInput").ap()
    wcat_d = nc.dram_tensor("wcat", [IN_F + 1, KS], BF16,
                            kind="ExternalInput").ap()
    # bf16 params: [wg2' | gamma' | beta'] (all (d,h)-permuted)
    parb_d = nc.dram_tensor("parb", [1, 3 * HD], BF16,
                            kind="ExternalInput").ap()
    # f32 params: [bgate, prelu_a, eps, pad]
    parf_d = nc.dram_tensor("parf", [1, 4], F32, kind="ExternalInput").ap()
    out_d = nc.dram_tensor("out", [P, G * HD], F32, kind="ExternalOutput").ap()

    # node-phase chunk boundaries; chunk j is emitted in the pipeline slot
    # right after run chunk_done[j]'s aggregation phase
    cw = [12, 12, 11, 9, 5]          # chunk sizes, small tail
    assert sum(cw) == G and len(cw) == NCHUNK
    cb = [sum(cw[:i]) for i in range(NCHUNK + 1)]
    chunk_done = {}
    for j in range(NCHUNK):
        i = min(i for i, (g0, g1, k) in enumerate(runs) if g1 >= cb[j + 1])
        chunk_done.setdefault(i, []).append((cb[j], cb[j + 1]))
    NGMAX = max(cb[j + 1] - cb[j] for j in range(NCHUNK))

    with tile.TileContext(nc) as tc:
        with (
            tc.tile_pool(name="singles", bufs=1) as singles,
            tc.tile_pool(name="plin", bufs=2, space="PSUM") as plin,
            tc.tile_pool(name="pscore", bufs=2, space="PSUM") as pscore,
            tc.tile_pool(name="pagg", bufs=2, space="PSUM") as pagg,
            tc.tile_pool(name="qvp", bufs=3) as qvp,
            tc.tile_pool(name="vvp", bufs=2) as vvp,
            tc.tile_pool(name="prodp", bufs=2) as prodp,
            tc.tile_pool(name="halfp", bufs=2) as halfp,
            tc.tile_pool(name="wp", bufs=2) as wp,
            tc.tile_pool(name="whp", bufs=2) as whp,
            tc.tile_pool(name="nodep", bufs=2) as nodep,
            tc.tile_pool(name="smallp", bufs=2) as smallp,
        ):
            qv_pre = {}
            vv_pre = {}

            def qv_fetch(i):
                g0, g1, K = runs[i]
                RK = (g1 - g0) * K
                r0 = int(rkbase[i])
                qt = qvp.tile([P, RKMAX * HD], BF16, tag="qt")
                in_ap = tabq_d[r0 * P:(r0 + RK) * P, :].rearrange(
                    "(p rk) e -> p (rk e)", p=P)
                nc.sync.dma_start(out=qt[:, :RK * HD], in_=in_ap)
                qv_pre[i] = qt

            def vv_fetch(i):
                g0, g1, K = runs[i]
                RK = (g1 - g0) * K
                r0 = int(rkbase[i])
                vt = vvp.tile([P, RKMAX * HD], BF16, tag="vt")
                in_ap = tabv_d[r0 * P:(r0 + RK) * P, :].rearrange(
                    "(p rk) e -> p (rk e)", p=P)
                nc.sync.dma_start(out=vt[:, :RK * HD], in_=in_ap)
                vv_pre[i] = vt

            # ---- static loads (featT split so linears start early);
            # the first featT quarter and wcat go ahead of the big edge-table
            # prefetches so the linear chain's dependencies land first
            featTs = []
            fg0 = 0
            for j, ng in enumerate(FT_G):
                t = singles.tile([IN_F + 1, ng * P], BF16, tag=f"ft{fg0}")
                nc.sync.dma_start(
                    out=t[:], in_=featT_d[:, fg0 * P:(fg0 + ng) * P])
                featTs.append((fg0, fg0 + ng, t))
                fg0 += ng
                if j == 0:
                    wcat = singles.tile([IN_F + 1, KS], BF16)
                    nc.sync.dma_start(out=wcat[:], in_=wcat_d[:])
                    qv_fetch(0)
                elif j == 1:
                    qv_fetch(1)

            def feat_slice(g):
                for a, b, t in featTs:
                    if a <= g < b:
                        return t[:, (g - a) * P:(g - a + 1) * P]
                raise AssertionError
            ident = singles.tile([P, P], BF16)
            nc.sync.dma_start(out=ident[:], in_=ident_d[:])
            padcnt = singles.tile([P, G * H], F32)
            nc.sync.dma_start(out=padcnt[:], in_=padc_d[:])
            parb = singles.tile([P, 3 * HD], BF16)
            nc.sync.dma_start(
                out=parb[:],
                in_=bass.AP(tensor=parb_d.tensor, offset=parb_d.offset,
                            ap=[[0, P], [1, 3 * HD]]))
            parf = singles.tile([P, 4], F32)
            nc.sync.dma_start(
                out=parf[:],
                in_=bass.AP(tensor=parf_d.tensor, offset=parf_d.offset,
                            ap=[[0, P], [1, 4]]))
            bg_ap = parf[:, 0:1]
            nbg_ap = parf[:, 3:4]
            pa_ap = parf[:, 1:2]
            eps_ap = parf[:, 2:3]

            # persistent state
            ks_bf = singles.tile([P, G * KS], BF16)   # k | skip | lgt | pad
            den = singles.tile([P, G * H], F32)
            agg_bf = singles.tile([P, G * HD], BF16)

            def pap(t, extra, off=0):
                sl = t[:, 0:1]
                return bass.AP(tensor=sl.tensor, offset=sl.offset + off,
                               ap=[sl.ap[0]] + extra)

            # ---- per-node linears: k|skip|lgt = featT_g.T @ wcat
            for c0 in range(0, G, 3):
                cn = min(3, G - c0)
                pl = plin.tile([P, 3 * KS], F32, tag="lin")
                for j in range(cn):
                    nc.tensor.matmul(out=pl[:, j * KS:(j + 1) * KS],
                                     lhsT=feat_slice(c0 + j),
                                     rhs=wcat[:], start=True, stop=True)
                nc.scalar.activation(out=ks_bf[:, c0 * KS:(c0 + cn) * KS],
                                     in_=pl[:, :cn * KS], func=ACTF.Copy)

            # ---- edge phase (software-pipelined emission)
            state = {}

            def score_phase(i):
                g0, g1, K = runs[i]
                R = g1 - g0
                RK = R * K
                r0 = int(rkbase[i])
                if i not in qv_pre:
                    qv_fetch(i)
                qt = qv_pre.pop(i)
                vv_fetch(i)

                # prod[rk, h, d] = q[rk, h, d] * k_g[h, d]  (bf16 2x)
                prod = prodp.tile([P, RKMAX * HD], BF16, tag="prod")
                q3 = pap(qt, [[HD * K, R], [HD, K], [1, HD]])
                kb = pap(ks_bf, [[KS, R], [0, K], [1, HD]], off=g0 * KS)
                p3 = pap(prod, [[HD * K, R], [HD, K], [1, HD]])
                nc.vector.tensor_tensor(out=p3, in0=q3, in1=kb, op=ALU.mult)

                # two pairwise pre-adds: 16 d-slices -> 4
                ph = halfp.tile([P, RKMAX * 32], BF16, tag="ph")
                nc.vector.tensor_tensor(
                    out=pap(ph, [[32, RK], [8, H], [1, 8]]),
                    in0=pap(prod, [[HD, RK], [D, H], [1, 8]]),
                    in1=pap(prod, [[HD, RK], [D, H], [1, 8]], off=8),
                    op=ALU.add)
                pq = halfp.tile([P, RKMAX * 16], BF16, tag="pq")
                nc.vector.tensor_tensor(
                    out=pap(pq, [[16, RK], [4, H], [1, 4]]),
                    in0=pap(ph, [[32, RK], [8, H], [1, 4]]),
                    in1=pap(ph, [[32, RK], [8, H], [1, 4]], off=4),
                    op=ALU.add)

                # score[rk, h] = sum_d4 pq  (PE accumulate)
                sp = pscore.tile([P, RKMAX * H], F32, tag="sp")
                for d in range(4):
                    rhs = pap(pq, [[16, RK], [4, H]], off=d)
                    nc.tensor.matmul(out=sp[:, :RK * H], lhsT=ident[:],
                                     rhs=rhs, start=(d == 0), stop=(d == 3))

                # w68 cols 64:68 = ex = exp(score/4)  (ACT, PSUM -> SBUF)
                w = wp.tile([P, RKMAX * W68], BF16, tag="w")
                nc.scalar.activation(
                    out=pap(w, [[W68, RK], [1, H]], off=HD),
                    in_=sp[:, :RK * H], func=ACTF.Exp, scale=0.25)
                state[i] = w

            def agg_phase(i):
                g0, g1, K = runs[i]
                R = g1 - g0
                RK = R * K
                w = state.pop(i)
                vt = vv_pre.pop(i)

                # w68[rk, 0:64] = v[rk, d, h] * ex[rk, h]  (bf16 2x)
                v3 = pap(vt, [[HD, RK], [H, D], [1, H]])
                eb = pap(w, [[W68, RK], [0, D], [1, H]], off=HD)
                w3 = pap(w, [[W68, RK], [H, D], [1, H]])
                nc.vector.tensor_tensor(out=w3, in0=v3, in1=eb, op=ALU.mult)

                # optional pairwise k pre-add for narrow runs
                ag = pagg.tile([P, RMAX * W68], F32, tag="agg")
                if R <= AGGPRE_R and K > 1:
                    KH = K // 2
                    wh = whp.tile([P, (RKMAX // 2 + 1) * W68], BF16, tag="wh")
                    nc.vector.tensor_tensor(
                        out=pap(wh, [[KH * W68, R], [W68, KH], [1, W68]]),
                        in0=pap(w, [[K * W68, R], [2 * W68, KH], [1, W68]]),
                        in1=pap(w, [[K * W68, R], [2 * W68, KH], [1, W68]],
                                off=W68),
                        op=ALU.add)
                    nmm = KH + (K % 2)
                    for k in range(KH):
                        rhs = pap(wh, [[KH * W68, R], [1, W68]], off=k * W68)
                        nc.tensor.matmul(out=ag[:, :R * W68], lhsT=ident[:],
                                         rhs=rhs, start=(k == 0),
                                         stop=(k == nmm - 1))
                    if K % 2:
                        rhs = pap(w, [[K * W68, R], [1, W68]],
                                  off=(K - 1) * W68)
                        nc.tensor.matmul(out=ag[:, :R * W68], lhsT=ident[:],
                                         rhs=rhs, start=(KH == 0), stop=True)
                else:
                    for k in range(K):
                        rhs = pap(w, [[K * W68, R], [1, W68]], off=k * W68)
                        nc.tensor.matmul(out=ag[:, :R * W68], lhsT=ident[:],
                                         rhs=rhs, start=(k == 0),
                                         stop=(k == K - 1))

                nc.scalar.activation(
                    out=pap(agg_bf, [[HD, R], [1, HD]], off=g0 * HD),
                    in_=pap(ag, [[W68, R], [1, HD]]),
                    func=ACTF.Copy)
                nc.scalar.activation(
                    out=pap(den, [[H, R], [1, H]], off=g0 * H),
                    in_=pap(ag, [[W68, R], [1, H]], off=HD),
                    func=ACTF.Copy)

            nstate = {}

            def node_chunk_a(h0, h1):
                NG = h1 - h0
                Fh = NG * HD
                bce = nc.vector if h1 == G else nc.gpsimd

                dv = smallp.tile([P, NGMAX * H], F32, tag="dinv")
                nc.vector.tensor_tensor(out=dv[:, :NG * H],
                                        in0=den[:, h0 * H:h1 * H],
                                        in1=padcnt[:, h0 * H:h1 * H],
                                        op=ALU.subtract)
                nc.vector.reciprocal_approx_fast(out=dv[:, :NG * H],
                                                 in_=dv[:, :NG * H])
                dvb = smallp.tile([P, NGMAX * H], BF16, tag="dinvb")
                nc.vector.tensor_copy(out=dvb[:, :NG * H],
                                      in_=dv[:, :NG * H])

                # rst = agg * dinv
                rst = nodep.tile([P, NGMAX * HD], BF16, tag="rst")
                dib = pap(dvb, [[H, NG], [0, D], [1, H]])
                a3 = pap(agg_bf, [[HD, NG], [H, D], [1, H]], off=h0 * HD)
                r3 = pap(rst, [[HD, NG], [H, D], [1, H]])
                nc.vector.tensor_tensor(out=r3, in0=a3, in1=dib, op=ALU.mult)

                # gate logit: lgt_skip (from linears) + sum_hd rst*wg2
                zc = nodep.tile([P, NGMAX * HD], BF16, tag="zc")
                sk = pap(ks_bf, [[KS, NG], [1, HD]], off=h0 * KS + HD)
                wg2 = pap(parb, [[0, NG], [1, HD]], off=0)
                nc.vector.tensor_tensor(out=zc[:, :Fh], in0=rst[:, :Fh],
                                        in1=wg2, op=ALU.mult)
                lgs = smallp.tile([P, NGMAX], F32, tag="lgs")
                nc.vector.tensor_reduce(
                    out=lgs[:, :NG],
                    in_=pap(zc, [[HD, NG], [1, HD]]),
                    axis=AX.X, op=ALU.add)
                logit = smallp.tile([P, NGMAX], F32, tag="logit")
                lgtb = pap(ks_bf, [[KS, NG], [1, 1]], off=h0 * KS + 2 * HD)
                nc.vector.tensor_tensor(out=logit[:, :NG], in0=lgs[:, :NG],
                                        in1=lgtb, op=ALU.add)
                ge = smallp.tile([P, NGMAX], F32, tag="ge")
                nc.scalar.activation(out=ge[:, :NG], in_=logit[:, :NG],
                                     func=ACTF.Exp, scale=-1.0, bias=nbg_ap)
                nc.vector.tensor_scalar(out=ge[:, :NG], in0=ge[:, :NG],
                                        scalar1=1.0, scalar2=None,
                                        op0=ALU.add)
                nc.vector.reciprocal_approx_fast(out=ge[:, :NG],
                                                 in_=ge[:, :NG])
                gate = smallp.tile([P, NGMAX], BF16, tag="gate")
                nc.vector.tensor_copy(out=gate[:, :NG], in_=ge[:, :NG])

                # rst += gate * (skip - rst)
                dif = nodep.tile([P, NGMAX * HD], BF16, tag="dif")
                nc.vector.tensor_tensor(out=dif[:, :Fh], in0=sk,
                                        in1=rst[:, :Fh], op=ALU.subtract)
                gb = pap(gate, [[1, NG], [0, HD]])
                d3 = pap(dif, [[HD, NG], [1, HD]])
                bce.tensor_tensor(out=d3, in0=d3, in1=gb, op=ALU.mult)
                nstate[h0] = (h1, rst, dif, bce)

            def node_chunk_bc(h0):
                h1, rst, dif, bce = nstate.pop(h0)
                NG = h1 - h0
                Fh = NG * HD
                nc.vector.tensor_tensor(out=rst[:, :Fh], in0=rst[:, :Fh],
                                        in1=dif[:, :Fh], op=ALU.add)

                # LayerNorm stats: sum rst (DVE), sum rst^2 (ACT square)
                sq = nodep.tile([P, NGMAX * HD], BF16, tag="zc2")
                nc.scalar.activation(out=sq[:, :Fh], in_=rst[:, :Fh],
                                     func=ACTF.Square)
                stats = smallp.tile([P, 2 * NGMAX], F32, tag="stats")
                nc.vector.tensor_reduce(
                    out=stats[:, :NG],
                    in_=pap(rst, [[HD, NG], [1, HD]]),
                    axis=AX.X, op=ALU.add)
                nc.vector.tensor_reduce(
                    out=stats[:, NG:2 * NG],
                    in_=pap(sq, [[HD, NG], [1, HD]]),
                    axis=AX.X, op=ALU.add)
                nc.vector.tensor_scalar(out=stats[:, :2 * NG],
                                        in0=stats[:, :2 * NG],
                                        scalar1=1.0 / HD, scalar2=None,
                                        op0=ALU.mult)
                mu = stats[:, 0:NG]
                msq = stats[:, NG:2 * NG]
                var = smallp.tile([P, NGMAX], F32, tag="var")
                nc.vector.tensor_tensor(out=var[:, :NG], in0=mu, in1=mu,
                                        op=ALU.mult)
                nc.vector.tensor_tensor(out=var[:, :NG], in0=msq,
                                        in1=var[:, :NG], op=ALU.subtract)
                sd = smallp.tile([P, NGMAX], F32, tag="sd")
                nc.scalar.activation(out=sd[:, :NG], in_=var[:, :NG],
                                     func=ACTF.Sqrt, bias=eps_ap)
                nc.vector.reciprocal_approx_fast(out=sd[:, :NG],
                                                 in_=sd[:, :NG])
                mrs = smallp.tile([P, 2 * NGMAX], BF16, tag="mrs")
                nc.vector.tensor_copy(out=mrs[:, :NG], in_=mu)
                nc.vector.tensor_copy(out=mrs[:, NG:2 * NG], in_=sd[:, :NG])

                # xhat = (rst - mu) * rstd; out = prelu(xhat*gamma + beta)
                mub = pap(mrs, [[1, NG], [0, HD]])
                bce.tensor_tensor(out=rst[:, :Fh], in0=rst[:, :Fh],
                                  in1=mub, op=ALU.subtract)
                rsb = pap(mrs, [[1, NG], [0, HD]], off=NG)
                bce.tensor_tensor(out=rst[:, :Fh], in0=rst[:, :Fh],
                                  in1=rsb, op=ALU.mult)
                gmb = pap(parb, [[0, NG], [1, HD]], off=HD)
                nc.vector.tensor_tensor(out=rst[:, :Fh], in0=rst[:, :Fh],
                                        in1=gmb, op=ALU.mult)
                btb = pap(parb, [[0, NG], [1, HD]], off=2 * HD)
                nc.vector.tensor_tensor(out=rst[:, :Fh], in0=rst[:, :Fh],
                                        in1=btb, op=ALU.add)
                outf = nodep.tile([P, NGMAX * HD], F32, tag="outf")
                nc.scalar.activation(out=outf[:, :Fh], in_=rst[:, :Fh],
                                     func=ACTF.Prelu, alpha=pa_ap)
                nc.sync.dma_start(out=out_d[:, h0 * HD:h1 * HD],
                                  in_=outf[:, :Fh])

            pend = []
            for i in range(nruns + 1):
                if i < nruns:
                    score_phase(i)
                for h0 in pend:
                    node_chunk_bc(h0)
                pend = []
                if i > 0:
                    agg_phase(i - 1)
                    for h0, h1 in chunk_done.get(i - 1, []):
                        node_chunk_a(h0, h1)
                        pend.append(h0)
            for h0 in pend:
                node_chunk_bc(h0)

    nc.compile()
    return nc


# ------------------------------------------------------------------- driver

_CACHE = {}


def _get_nc(plan, ncores):
    key = (tuple(int(k) for g0, g1, k in plan["runs"]),
           tuple(g1 - g0 for g0, g1, k in plan["runs"]),
           plan["grid"], ncores)
    if key not in _CACHE:
        _CACHE[key] = _build_nc(plan, ncores)
    return _CACHE[key]


def _make_inmaps(plan, params, ncores):
    (Wk, bk, Wskip, bskip, Wgate, bgate, ln_gamma, ln_beta, prelu_a) = params
    Wk = np.asarray(Wk, np.float32)
    bk = np.asarray(bk, np.float32)
    Wsk = np.asarray(Wskip, np.float32)
    bsk = np.asarray(bskip, np.float32)
    wg = np.asarray(Wgate, np.float32).reshape(3 * HD)
    wg1n = wg[0:HD] + wg[2 * HD:3 * HD]                 # acts on skip (nat)
    wcat = np.zeros((IN_F + 1, KS), np.float32)
    wcat[:IN_F, :HD] = Wk
    wcat[IN_F, :HD] = bk
    wcat[:IN_F, HD:2 * HD] = Wsk[:, _PERM]
    wcat[IN_F, HD:2 * HD] = bsk[_PERM]
    wcat[:IN_F, 2 * HD] = Wsk @ wg1n                    # skip-side gate logit
    wcat[IN_F, 2 * HD] = bsk @ wg1n
    wcat = wcat.astype(BF)

    parb = np.zeros((1, 3 * HD), np.float32)
    parb[0, 0:HD] = (wg[HD:2 * HD] - wg[2 * HD:3 * HD])[_PERM]   # on rst
    parb[0, HD:2 * HD] = np.asarray(ln_gamma, np.float32)[_PERM]
    parb[0, 2 * HD:3 * HD] = np.asarray(ln_beta, np.float32)[_PERM]
    parb = parb.astype(BF)
    parf = np.zeros((1, 4), np.float32)
    parf[0, 0] = np.float32(np.asarray(bgate).reshape(-1)[0])
    parf[0, 1] = np.float32(np.asarray(prelu_a).reshape(-1)[0])
    parf[0, 2] = 1e-5
    parf[0, 3] = -parf[0, 0]

    in_maps = []
    for c in range(ncores):
        pc = plan["per_core"][c]
        m = dict(featT=plan["featTs"][c], tabq=pc["tabq"],
                 tabv=pc["tabv"], padcnt=pc["padcnt"], ident=plan["ident"],
                 wcat=wcat, parb=parb, parf=parf)
        in_maps.append(m)
    return in_maps


def run(q_src, v_src, feat, src, dst, Wk, bk, Wskip, bskip, Wgate, bgate,
        ln_gamma, ln_beta, prelu_a, ncores=NCORES, trace=False):
    plan = _plan(q_src, v_src, feat, src, dst, ncores)
    nc = _get_nc(plan, ncores)
    in_maps = _make_inmaps(
        plan, (Wk, bk, Wskip, bskip, Wgate, bgate, ln_gamma, ln_beta, prelu_a),
        ncores)
    res = run_bass_kernel_spmd(nc, in_maps, core_ids=list(range(ncores)),
                               trace=trace)
    n, npc, ngrp = plan["n"], plan["npc"], plan["ngrp"]
    out = np.empty((n, HD), np.float32)
    for c in range(ncores):
        r = np.asarray(res.results[c]["out"])              # [128, ngrp*64]
        r = r.reshape(P, ngrp, D, H).transpose(1, 0, 3, 2)  # -> [g, p, h, d]
        arr = r.reshape(-1, HD)
        out[c * npc + plan["cores"][c]["perm"]] = \
            arr[plan["ndum"]:plan["ndum"] + npc]
    return out, res, plan, in_maps, nc


def kernel(**inputs):
    out, _, _, _, _ = run(**inputs)
    return out


# revision 28
# speedup vs baseline: 1.1090x; 1.0146x over previous
"""Trainium2 Bass kernel for nn_DenTargetTransformerConv (GNN message passing).

Strategy (graph/data parallel, dst-owner sharding across 8 NeuronCores):
  - Nodes are partitioned by dst-id range; each core owns N/8 nodes and all
    edges whose dst falls in its range. The "halo exchange" of src features is
    materialized host-side as per-core edge-slot tables (rows replicated per
    consumer), so the device reads are plain strided DMAs.
  - Per core, own nodes are sorted by in-degree and packed into groups of 128
    (SBUF partition dim). Consecutive groups are merged into equal-K runs
    (K = slots per node, shared across the 8 cores so one program serves all).
  - Edge phase per run: one static DMA brings the [128, RK*128] bf16 q||v
    slot tile; DVE computes per-slot q*k products and exp-weighted v in bf16
    (2x mode); the segment reductions (score over D, aggregation over K) run
    on the Tensor engine as identity-weight PSUM-accumulate matmul chains
    (with one DVE pairwise pre-add stage in front to halve the chain length
    where that balances the engines), and the softmax pad-mask is folded in
    as one extra accumulated matmul of a -400 bias table. exp runs on the
    Scalar engine straight out of PSUM and writes its results interleaved
    into the weighted-v tile, so the softmax denominators fall out of the
    aggregation matmuls as 4 extra PSUM columns per group.
  - v (and everything downstream of the aggregation) lives in a (d,h)
    interleaved layout so the exp broadcast has a step-1 inner axis (DVE 2x
    mode); the host un-permutes the final output.
  - The gate's skip-side logit is a linear function of feat, so it is folded
    into the per-node linears as one extra matmul column. The node phase
    (softmax normalize, gate, LayerNorm, PReLU) runs in three group-chunks
    interleaved with the edge runs; broadcast multiplies go to GpSimd,
    transcendentals to the Scalar engine.
  - Emission is software-pipelined: run i's score phase is emitted before
    run i-1's weighted-aggregation phase, so the in-order DVE stream never
    stalls waiting for a PE/ACT round trip.
"""

import numpy as np
import ml_dtypes

import concourse.bacc as bacc
import concourse.bass as bass
import concourse.tile as tile
from concourse import mybir
from concourse.bass_utils import run_bass_kernel_spmd

F32 = mybir.dt.float32
BF16 = mybir.dt.bfloat16
AX = mybir.AxisListType
ALU = mybir.AluOpType
ACTF = mybir.ActivationFunctionType
BF = ml_dtypes.bfloat16

P = 128
NCORES = 8
HD = 64          # H * D
H, D = 4, 16
IN_F = 64
W68 = HD + H     # weighted-v row + denominator columns
KS = 2 * HD + 2  # per-group ks row: k(64) | skip(64) | lgt | pad

RMAX = 7         # max groups per run (agg PSUM: R*68 <= 476)
RKMAX = 96       # max slot-columns per run
KSPREAD = 2     # max K padding when merging groups into a run
NCHUNK = 5       # node-phase chunks
AGGPRE_R = 2     # agg pairwise pre-add for runs with R <= this

# natural hd = h*16+d  <->  stored j = d*4+h
_PERM = np.arange(HD).reshape(H, D).T.reshape(-1)       # j -> natural hd


# ----------------------------------------------------------------- host prep

def _plan(q_src, v_src, feat, src, dst, ncores):
    n = feat.shape[0]
    npc = n // ncores
    ngrp = (npc + P - 1) // P
    grid = ngrp * P
    ndum = grid - npc

    q2 = np.asarray(q_src, np.float32).reshape(n, HD)
    v2 = np.asarray(v_src, np.float32).reshape(n, H, D).transpose(0, 2, 1).reshape(n, HD)
    qv = np.concatenate([q2, v2], axis=1).astype(BF)    # [n, 128]

    src = np.asarray(src).astype(np.int64)
    dst = np.asarray(dst).astype(np.int64)
    order = np.argsort(dst, kind="stable")
    dst_s, src_s = dst[order], src[order]
    bounds = np.searchsorted(dst_s, np.arange(ncores + 1) * npc)

    cores = []
    gmax = np.zeros((ncores, ngrp), np.int64)
    for c in range(ncores):
        lo, hi = bounds[c], bounds[c + 1]
        dstL = dst_s[lo:hi] - c * npc          # ascending
        srcL = src_s[lo:hi]
        deg = np.bincount(dstL, minlength=npc)
        starts = np.concatenate([[0], np.cumsum(deg)])
        rank = np.arange(len(dstL)) - starts[dstL]
        perm = np.argsort(deg, kind="stable")  # ascending degree
        pos_of = np.empty(npc, np.int64)
        pos_of[perm] = ndum + np.arange(npc)
        gd = np.zeros(grid, np.int64)
        gd[ndum:] = deg[perm]
        gmax[c] = gd.reshape(ngrp, P).max(1)
        cores.append(dict(dstL=dstL, srcL=srcL, rank=rank, perm=perm,
                          pos_of=pos_of))

    K = np.maximum(gmax.max(0), 1)             # per-group slot count

    # merge consecutive groups into equal-K runs (pad K up to the run max)
    runs = []
    g = 0
    while g < ngrp:
        ge = g + 1
        while (ge < ngrp and ge - g < RMAX
               and (ge - g + 1) * K[ge] <= RKMAX
               and K[ge] - K[g] <= KSPREAD):
            ge += 1
        runs.append((g, ge, int(K[ge - 1])))
        g = ge
    rkbase = np.zeros(len(runs) + 1, np.int64)
    for i, (g0, g1, k) in enumerate(runs):
        rkbase[i + 1] = rkbase[i] + (g1 - g0) * k
    totrk = int(rkbase[-1])

    # per-core tables
    per_core = []
    grp_run = np.zeros(ngrp, np.int64)
    for i, (g0, g1, k) in enumerate(runs):
        grp_run[g0:g1] = i
    run_g0 = np.array([r[0] for r in runs])
    run_k = np.array([r[2] for r in runs])

    for c in range(ncores):
        cd = cores[c]
        pos_e = cd["pos_of"][cd["dstL"]]       # grid position of each edge
        g_e = pos_e // P
        p_e = pos_e % P
        i_e = grp_run[g_e]
        r_e = g_e - run_g0[i_e]
        k_e = run_k[i_e]
        # row = rkbase[i]*128 + p*(R*K) + r*K + rank  (partition-major)
        rk_run = np.array([r[1] - r[0] for r in runs])[i_e] * k_e
        rows = rkbase[i_e] * P + p_e * rk_run + r_e * k_e + cd["rank"]
        tabq = np.zeros((totrk * P, HD), BF)
        tabv = np.zeros((totrk * P, HD), BF)
        tabq[rows] = qv[cd["srcL"], :HD]
        tabv[rows] = qv[cd["srcL"], HD:]
        # padded slots have q=v=0 -> score 0 -> exp 1; count them per node
        # so the denominator can be corrected (eps folded in)
        nslot = np.zeros(ngrp, np.int64)
        for i, (g0, g1, k) in enumerate(runs):
            nslot[g0:g1] = k
        real = np.zeros((P, ngrp), np.float32)
        np.add.at(real, (p_e, g_e), 1.0)
        padc = nslot[None, :] - real - 1e-9
        padcnt = np.repeat(padc, H, axis=1).astype(np.float32)  # [128, G*4]
        per_core.append(dict(tabq=tabq, tabv=tabv, padcnt=padcnt))

    # featT with ones row, per core, grid-permuted: [IN_F+1, grid] bf16
    featTs = []
    feat = np.asarray(feat, np.float32)
    for c in range(ncores):
        ft = np.zeros((IN_F + 1, grid), np.float32)
        ft[IN_F, :] = 1.0
        perm = cores[c]["perm"]
        ft[:IN_F, ndum:] = feat[c * npc + perm].T
        featTs.append(ft.astype(BF))

    ident = np.eye(P, dtype=BF)

    return dict(n=n, npc=npc, ngrp=ngrp, grid=grid, ndum=ndum, K=K,
                runs=runs, rkbase=rkbase, totrk=totrk, ident=ident,
                cores=cores, per_core=per_core, featTs=featTs)


# ------------------------------------------------------------- device build

def _build_nc(plan, ncores):
    ngrp, runs, rkbase, totrk = (plan["ngrp"], plan["runs"], plan["rkbase"],
                                 plan["totrk"])
    grid = plan["grid"]
    G = ngrp
    nruns = len(runs)

    nc = bacc.Bacc("TRN2", target_bir_lowering=False, debug=False,
                   num_devices=ncores)

    featT_d = nc.dram_tensor("featT", [IN_F + 1, grid], BF16,
                             kind="ExternalInput").ap()
    FT_G = [12, 12, 12, G - 36]  # featT split sizes (groups)
    tabq_d = nc.dram_tensor("tabq", [totrk * P, HD], BF16,
                            kind="ExternalInput").ap()
    tabv_d = nc.dram_tensor("tabv", [totrk * P, HD], BF16,
                            kind="ExternalInput").ap()
    padc_d = nc.dram_tensor("padcnt", [P, G * H], F32,
                            kind="ExternalInput").ap()
    ident_d = nc.dram_tensor("ident", [P, P], BF16, kind="ExternalInput").ap()
    wcat_d = nc.dram_tensor("wcat", [IN_F + 1, KS], BF16,
                            kind="ExternalInput").ap()
    # bf16 params: [wg2' | gamma' | beta'] (all (d,h)-permuted)
    parb_d = nc.dram_tensor("parb", [1, 3 * HD], BF16,
                            kind="ExternalInput").ap()
    # f32 params: [bgate, prelu_a, eps, pad]
    parf_d = nc.dram_tensor("parf", [1, 4], F32, kind="ExternalInput").ap()
    out_d = nc.dram_tensor("out", [P, G * HD], F32, kind="ExternalOutput").ap()

    # node-phase chunk boundaries; chunk j is emitted in the pipeline slot
    # right after run chunk_done[j]'s aggregation phase
    cw = [12, 12, 11, 9, 5]          # chunk sizes, small tail
    assert sum(cw) == G and len(cw) == NCHUNK
    cb = [sum(cw[:i]) for i in range(NCHUNK + 1)]
    chunk_done = {}
    for j in range(NCHUNK):
        i = min(i for i, (g0, g1, k) in enumerate(runs) if g1 >= cb[j + 1])
        chunk_done.setdefault(i, []).append((cb[j], cb[j + 1]))
    NGMAX = max(cb[j + 1] - cb[j] for j in range(NCHUNK))

    with tile.TileContext(nc) as tc:
        with (
            tc.tile_pool(name="singles", bufs=1) as singles,
            tc.tile_pool(name="plin", bufs=2, space="PSUM") as plin,
            tc.tile_pool(name="pscore", bufs=2, space="PSUM") as pscore,
            tc.tile_pool(name="pagg", bufs=2, space="PSUM") as pagg,
            tc.tile_pool(name="qvp", bufs=3) as qvp,
            tc.tile_pool(name="vvp", bufs=2) as vvp,
            tc.tile_pool(name="prodp", bufs=2) as prodp,
            tc.tile_pool(name="halfp", bufs=2) as halfp,
            tc.tile_pool(name="wp", bufs=2) as wp,
            tc.tile_pool(name="whp", bufs=2) as whp,
            tc.tile_pool(name="nodep", bufs=2) as nodep,
            tc.tile_pool(name="smallp", bufs=2) as smallp,
        ):
            qv_pre = {}
            vv_pre = {}

            def qv_fetch(i):
                g0, g1, K = runs[i]
                RK = (g1 - g0) * K
                r0 = int(rkbase[i])
                qt = qvp.tile([P, RKMAX * HD], BF16, tag="qt")
                in_ap = tabq_d[r0 * P:(r0 + RK) * P, :].rearrange(
                    "(p rk) e -> p (rk e)", p=P)
                nc.sync.dma_start(out=qt[:, :RK * HD], in_=in_ap)
                qv_pre[i] = qt

            def vv_fetch(i):
                g0, g1, K = runs[i]
                RK = (g1 - g0) * K
                r0 = int(rkbase[i])
                vt = vvp.tile([P, RKMAX * HD], BF16, tag="vt")
                in_ap = tabv_d[r0 * P:(r0 + RK) * P, :].rearrange(
                    "(p rk) e -> p (rk e)", p=P)
                nc.sync.dma_start(out=vt[:, :RK * HD], in_=in_ap)
                vv_pre[i] = vt

            # ---- static loads (featT split so linears start early);
            # the first featT quarter and wcat go ahead of the big edge-table
            # prefetches so the linear chain's dependencies land first
            featTs = []
            fg0 = 0
            for j, ng in enumerate(FT_G):
                t = singles.tile([IN_F + 1, ng * P], BF16, tag=f"ft{fg0}")
                nc.sync.dma_start(
                    out=t[:], in_=featT_d[:, fg0 * P:(fg0 + ng) * P])
                featTs.append((fg0, fg0 + ng, t))
                fg0 += ng
                if j == 0:
                    wcat = singles.tile([IN_F + 1, KS], BF16)
                    nc.sync.dma_start(out=wcat[:], in_=wcat_d[:])
                    qv_fetch(0)
                elif j == 1:
                    qv_fetch(1)

            def feat_slice(g):
                for a, b, t in featTs:
                    if a <= g < b:
                        return t[:, (g - a) * P:(g - a + 1) * P]
                raise AssertionError
            ident = singles.tile([P, P], BF16)
            nc.sync.dma_start(out=ident[:], in_=ident_d[:])
            padcnt = singles.tile([P, G * H], F32)
            nc.sync.dma_start(out=padcnt[:], in_=padc_d[:])
            parb = singles.tile([P, 3 * HD], BF16)
            nc.sync.dma_start(
                out=parb[:],
                in_=bass.AP(tensor=parb_d.tensor, offset=parb_d.offset,
                            ap=[[0, P], [1, 3 * HD]]))
            parf = singles.tile([P, 4], F32)
            nc.sync.dma_start(
                out=parf[:],
                in_=bass.AP(tensor=parf_d.tensor, offset=parf_d.offset,
                            ap=[[0, P], [1, 4]]))
            bg_ap = parf[:, 0:1]
            nbg_ap = parf[:, 3:4]
            pa_ap = parf[:, 1:2]
            eps_ap = parf[:, 2:3]

            # persistent state
            ks_bf = singles.tile([P, G * KS], BF16)   # k | skip | lgt | pad
            den = singles.tile([P, G * H], F32)
            agg_bf = singles.tile([P, G * HD], BF16)

            def pap(t, extra, off=0):
                sl = t[:, 0:1]
                return bass.AP(tensor=sl.tensor, offset=sl.offset + off,
                               ap=[sl.ap[0]] + extra)

            # ---- per-node linears: k|skip|lgt = featT_g.T @ wcat
            for c0 in range(0, G, 3):
                cn = min(3, G - c0)
                pl = plin.tile([P, 3 * KS], F32, tag="lin")
                for j in range(cn):
                    nc.tensor.matmul(out=pl[:, j * KS:(j + 1) * KS],
                                     lhsT=feat_slice(c0 + j),
                                     rhs=wcat[:], start=True, stop=True)
                nc.scalar.activation(out=ks_bf[:, c0 * KS:(c0 + cn) * KS],
                                     in_=pl[:, :cn * KS], func=ACTF.Copy)

            # ---- edge phase (software-pipelined emission)
            state = {}

            def score_phase(i):
                g0, g1, K = runs[i]
                R = g1 - g0
                RK = R * K
                r0 = int(rkbase[i])
                if i not in qv_pre:
                    qv_fetch(i)
                qt = qv_pre.pop(i)
                vv_fetch(i)

                # prod[rk, h, d] = q[rk, h, d] * k_g[h, d]  (bf16 2x)
                prod = prodp.tile([P, RKMAX * HD], BF16, tag="prod")
                q3 = pap(qt, [[HD * K, R], [HD, K], [1, HD]])
                kb = pap(ks_bf, [[KS, R], [0, K], [1, HD]], off=g0 * KS)
                p3 = pap(prod, [[HD * K, R], [HD, K], [1, HD]])
                nc.vector.tensor_tensor(out=p3, in0=q3, in1=kb, op=ALU.mult)

                # two pairwise pre-adds: 16 d-slices -> 4
                ph = halfp.tile([P, RKMAX * 32], BF16, tag="ph")
                nc.vector.tensor_tensor(
                    out=pap(ph, [[32, RK], [8, H], [1, 8]]),
                    in0=pap(prod, [[HD, RK], [D, H], [1, 8]]),
                    in1=pap(prod, [[HD, RK], [D, H], [1, 8]], off=8),
                    op=ALU.add)
                pq = halfp.tile([P, RKMAX * 16], BF16, tag="pq")
                nc.vector.tensor_tensor(
                    out=pap(pq, [[16, RK], [4, H], [1, 4]]),
                    in0=pap(ph, [[32, RK], [8, H], [1, 4]]),
                    in1=pap(ph, [[32, RK], [8, H], [1, 4]], off=4),
                    op=ALU.add)

                # score[rk, h] = sum_d4 pq  (PE accumulate)
                sp = pscore.tile([P, RKMAX * H], F32, tag="sp")
                for d in range(4):
                    rhs = pap(pq, [[16, RK], [4, H]], off=d)
                    nc.tensor.matmul(out=sp[:, :RK * H], lhsT=ident[:],
                                     rhs=rhs, start=(d == 0), stop=(d == 3))

                # w68 cols 64:68 = ex = exp(score/4)  (ACT, PSUM -> SBUF)
                w = wp.tile([P, RKMAX * W68], BF16, tag="w")
                nc.scalar.activation(
                    out=pap(w, [[W68, RK], [1, H]], off=HD),
                    in_=sp[:, :RK * H], func=ACTF.Exp, scale=0.25)
                state[i] = w

            def agg_phase(i):
                g0, g1, K = runs[i]
                R = g1 - g0
                RK = R * K
                w = state.pop(i)
                vt = vv_pre.pop(i)

                # w68[rk, 0:64] = v[rk, d, h] * ex[rk, h]  (bf16 2x)
                v3 = pap(vt, [[HD, RK], [H, D], [1, H]])
                eb = pap(w, [[W68, RK], [0, D], [1, H]], off=HD)
                w3 = pap(w, [[W68, RK], [H, D], [1, H]])
                nc.vector.tensor_tensor(out=w3, in0=v3, in1=eb, op=ALU.mult)

                # optional pairwise k pre-add for narrow runs
                ag = pagg.tile([P, RMAX * W68], F32, tag="agg")
                if R <= AGGPRE_R and K > 1:
                    KH = K // 2
                    wh = whp.tile([P, (RKMAX // 2 + 1) * W68], BF16, tag="wh")
                    nc.vector.tensor_tensor(
                        out=pap(wh, [[KH * W68, R], [W68, KH], [1, W68]]),
                        in0=pap(w, [[K * W68, R], [2 * W68, KH], [1, W68]]),
                        in1=pap(w, [[K * W68, R], [2 * W68, KH], [1, W68]],
                                off=W68),
                        op=ALU.add)
                    nmm = KH + (K % 2)
                    for k in range(KH):
                        rhs = pap(wh, [[KH * W68, R], [1, W68]], off=k * W68)
                        nc.tensor.matmul(out=ag[:, :R * W68], lhsT=ident[:],
                                         rhs=rhs, start=(k == 0),
                                         stop=(k == nmm - 1))
                    if K % 2:
                        rhs = pap(w, [[K * W68, R], [1, W68]],
                                  off=(K - 1) * W68)
                        nc.tensor.matmul(out=ag[:, :R * W68], lhsT=ident[:],
                                         rhs=rhs, start=(KH == 0), stop=True)
                else:
                    for k in range(K):
                        rhs = pap(w, [[K * W68, R], [1, W68]], off=k * W68)
                        nc.tensor.matmul(out=ag[:, :R * W68], lhsT=ident[:],
                                         rhs=rhs, start=(k == 0),
                                         stop=(k == K - 1))

                nc.scalar.activation(
                    out=pap(agg_bf, [[HD, R], [1, HD]], off=g0 * HD),
                    in_=pap(ag, [[W68, R], [1, HD]]),
                    func=ACTF.Copy)
                nc.scalar.activation(
                    out=pap(den, [[H, R], [1, H]], off=g0 * H),
                    in_=pap(ag, [[W68, R], [1, H]], off=HD),
                    func=ACTF.Copy)

            nstate = {}

            def node_chunk_a(h0, h1):
                NG = h1 - h0
                Fh = NG * HD
                bce = nc.vector if h1 == G else nc.gpsimd

                dv = smallp.tile([P, NGMAX * H], F32, tag="dinv")
                nc.vector.tensor_tensor(out=dv[:, :NG * H],
                                        in0=den[:, h0 * H:h1 * H],
                                        in1=padcnt[:, h0 * H:h1 * H],
                                        op=ALU.subtract)
                nc.vector.reciprocal_approx_fast(out=dv[:, :NG * H],
                                                 in_=dv[:, :NG * H])
                dvb = smallp.tile([P, NGMAX * H], BF16, tag="dinvb")
                nc.vector.tensor_copy(out=dvb[:, :NG * H],
                                      in_=dv[:, :NG * H])

                # rst = agg * dinv
                rst = nodep.tile([P, NGMAX * HD], BF16, tag="rst")
                dib = pap(dvb, [[H, NG], [0, D], [1, H]])
                a3 = pap(agg_bf, [[HD, NG], [H, D], [1, H]], off=h0 * HD)
                r3 = pap(rst, [[HD, NG], [H, D], [1, H]])
                nc.vector.tensor_tensor(out=r3, in0=a3, in1=dib, op=ALU.mult)

                # gate logit: lgt_skip (from linears) + sum_hd rst*wg2
                zc = nodep.tile([P, NGMAX * HD], BF16, tag="zc")
                sk = pap(ks_bf, [[KS, NG], [1, HD]], off=h0 * KS + HD)
                wg2 = pap(parb, [[0, NG], [1, HD]], off=0)
                nc.vector.tensor_tensor(out=zc[:, :Fh], in0=rst[:, :Fh],
                                        in1=wg2, op=ALU.mult)
                lgs = smallp.tile([P, NGMAX], F32, tag="lgs")
                nc.vector.tensor_reduce(
                    out=lgs[:, :NG],
                    in_=pap(zc, [[HD, NG], [1, HD]]),
                    axis=AX.X, op=ALU.add)
                logit = smallp.tile([P, NGMAX], F32, tag="logit")
                lgtb = pap(ks_bf, [[KS, NG], [1, 1]], off=h0 * KS + 2 * HD)
                nc.vector.tensor_tensor(out=logit[:, :NG], in0=lgs[:, :NG],
                                        in1=lgtb, op=ALU.add)
                ge = smallp.tile([P, NGMAX], F32, tag="ge")
                nc.scalar.activation(out=ge[:, :NG], in_=logit[:, :NG],
                                     func=ACTF.Exp, scale=-1.0, bias=nbg_ap)
                nc.vector.tensor_scalar(out=ge[:, :NG], in0=ge[:, :NG],
                                        scalar1=1.0, scalar2=None,
                                        op0=ALU.add)
                nc.vector.reciprocal_approx_fast(out=ge[:, :NG],
                                                 in_=ge[:, :NG])
                gate = smallp.tile([P, NGMAX], BF16, tag="gate")
                nc.vector.tensor_copy(out=gate[:, :NG], in_=ge[:, :NG])

                # rst += gate * (skip - rst)
                dif = nodep.tile([P, NGMAX * HD], BF16, tag="dif")
                nc.vector.tensor_tensor(out=dif[:, :Fh], in0=sk,
                                        in1=rst[:, :Fh], op=ALU.subtract)
                gb = pap(gate, [[1, NG], [0, HD]])
                d3 = pap(dif, [[HD, NG], [1, HD]])
                bce.tensor_tensor(out=d3, in0=d3, in1=gb, op=ALU.mult)
                nstate[h0] = (h1, rst, dif, bce)

            def node_chunk_bc(h0):
                h1, rst, dif, bce = nstate.pop(h0)
                NG = h1 - h0
                Fh = NG * HD
                nc.vector.tensor_tensor(out=rst[:, :Fh], in0=rst[:, :Fh],
                                        in1=dif[:, :Fh], op=ALU.add)

                # LayerNorm stats: sum rst (DVE), sum rst^2 (ACT square)
                sq = nodep.tile([P, NGMAX * HD], BF16, tag="zc2")
                nc.scalar.activation(out=sq[:, :Fh], in_=rst[:, :Fh],
                                     func=ACTF.Square)
                stats = smallp.tile([P, 2 * NGMAX], F32, tag="stats")
                nc.vector.tensor_reduce(
                    out=stats[:, :NG],
                    in_=pap(rst, [[HD, NG], [1, HD]]),
                    axis=AX.X, op=ALU.add)
                nc.vector.tensor_reduce(
                    out=stats[:, NG:2 * NG],
                    in_=pap(sq, [[HD, NG], [1, HD]]),
                    axis=AX.X, op=ALU.add)
                nc.vector.tensor_scalar(out=stats[:, :2 * NG],
                                        in0=stats[:, :2 * NG],
                                        scalar1=1.0 / HD, scalar2=None,
                                        op0=ALU.mult)
                mu = stats[:, 0:NG]
                msq = stats[:, NG:2 * NG]
                var = smallp.tile([P, NGMAX], F32, tag="var")
                nc.vector.tensor_tensor(out=var[:, :NG], in0=mu, in1=mu,
                                        op=ALU.mult)
                nc.vector.tensor_tensor(out=var[:, :NG], in0=msq,
                                        in1=var[:, :NG], op=ALU.subtract)
                sd = smallp.tile([P, NGMAX], F32, tag="sd")
                nc.scalar.activation(out=sd[:, :NG], in_=var[:, :NG],
                                     func=ACTF.Sqrt, bias=eps_ap)
                nc.vector.reciprocal_approx_fast(out=sd[:, :NG],
                                                 in_=sd[:, :NG])
                mrs = smallp.tile([P, 2 * NGMAX], BF16, tag="mrs")
                nc.vector.tensor_copy(out=mrs[:, :NG], in_=mu)
                nc.vector.tensor_copy(out=mrs[:, NG:2 * NG], in_=sd[:, :NG])

                # xhat = (rst - mu) * rstd; out = prelu(xhat*gamma + beta)
                mub = pap(mrs, [[1, NG], [0, HD]])
                bce.tensor_tensor(out=rst[:, :Fh], in0=rst[:, :Fh],
                                  in1=mub, op=ALU.subtract)
                rsb = pap(mrs, [[1, NG], [0, HD]], off=NG)
                bce.tensor_tensor(out=rst[:, :Fh], in0=rst[:, :Fh],
                                  in1=rsb, op=ALU.mult)
                gmb = pap(parb, [[0, NG], [1, HD]], off=HD)
                nc.vector.tensor_tensor(out=rst[:, :Fh], in0=rst[:, :Fh],
                                        in1=gmb, op=ALU.mult)
                btb = pap(parb, [[0, NG], [1, HD]], off=2 * HD)
                nc.vector.tensor_tensor(out=rst[:, :Fh], in0=rst[:, :Fh],
                                        in1=btb, op=ALU.add)
                outf = nodep.tile([P, NGMAX * HD], F32, tag="outf")
                nc.scalar.activation(out=outf[:, :Fh], in_=rst[:, :Fh],
                                     func=ACTF.Prelu, alpha=pa_ap)
                nc.sync.dma_start(out=out_d[:, h0 * HD:h1 * HD],
                                  in_=outf[:, :Fh])

            pend = []
            for i in range(nruns + 1):
                if i < nruns:
                    score_phase(i)
                for h0 in pend:
                    node_chunk_bc(h0)
                pend = []
                if i > 0:
                    agg_phase(i - 1)
                    for h0, h1 in chunk_done.get(i - 1, []):
                        node_chunk_a(h0, h1)
                        pend.append(h0)
            for h0 in pend:
                node_chunk_bc(h0)

    nc.compile()
    return nc


# ------------------------------------------------------------------- driver

_CACHE = {}


def _get_nc(plan, ncores):
    key = (tuple(int(k) for g0, g1, k in plan["runs"]),
           tuple(g1 - g0 for g0, g1, k in plan["runs"]),
           plan["grid"], ncores)
    if key not in _CACHE:
        _CACHE[key] = _build_nc(plan, ncores)
    return _CACHE[key]


def _make_inmaps(plan, params, ncores):
    (Wk, bk, Wskip, bskip, Wgate, bgate, ln_gamma, ln_beta, prelu_a) = params
    Wk = np.asarray(Wk, np.float32)
    bk = np.asarray(bk, np.float32)
    Wsk = np.asarray(Wskip, np.float32)
    bsk = np.asarray(bskip, np.float32)
    wg = np.asarray(Wgate, np.float32).reshape(3 * HD)
    wg1n = wg[0:HD] + wg[2 * HD:3 * HD]                 # acts on skip (nat)
    wcat = np.zeros((IN_F + 1, KS), np.float32)
    wcat[:IN_F, :HD] = Wk
    wcat[IN_F, :HD] = bk
    wcat[:IN_F, HD:2 * HD] = Wsk[:, _PERM]
    wcat[IN_F, HD:2 * HD] = bsk[_PERM]
    wcat[:IN_F, 2 * HD] = Wsk @ wg1n                    # skip-side gate logit
    wcat[IN_F, 2 * HD] = bsk @ wg1n
    wcat = wcat.astype(BF)

    parb = np.zeros((1, 3 * HD), np.float32)
    parb[0, 0:HD] = (wg[HD:2 * HD] - wg[2 * HD:3 * HD])[_PERM]   # on rst
    parb[0, HD:2 * HD] = np.asarray(ln_gamma, np.float32)[_PERM]
    parb[0, 2 * HD:3 * HD] = np.asarray(ln_beta, np.float32)[_PERM]
    parb = parb.astype(BF)
    parf = np.zeros((1, 4), np.float32)
    parf[0, 0] = np.float32(np.asarray(bgate).reshape(-1)[0])
    parf[0, 1] = np.float32(np.asarray(prelu_a).reshape(-1)[0])
    parf[0, 2] = 1e-5
    parf[0, 3] = -parf[0, 0]

    in_maps = []
    for c in range(ncores):
        pc = plan["per_core"][c]
        m = dict(featT=plan["featTs"][c], tabq=pc["tabq"],
                 tabv=pc["tabv"], padcnt=pc["padcnt"], ident=plan["ident"],
                 wcat=wcat, parb=parb, parf=parf)
        in_maps.append(m)
    return in_maps


def run(q_src, v_src, feat, src, dst, Wk, bk, Wskip, bskip, Wgate, bgate,
        ln_gamma, ln_beta, prelu_a, ncores=NCORES, trace=False):
    plan = _plan(q_src, v_src, feat, src, dst, ncores)
    nc = _get_nc(plan, ncores)
    in_maps = _make_inmaps(
        plan, (Wk, bk, Wskip, bskip, Wgate, bgate, ln_gamma, ln_beta, prelu_a),
        ncores)
    res = run_bass_kernel_spmd(nc, in_maps, core_ids=list(range(ncores)),
                               trace=trace)
    n, npc, ngrp = plan["n"], plan["npc"], plan["ngrp"]
    out = np.empty((n, HD), np.float32)
    for c in range(ncores):
        r = np.asarray(res.results[c]["out"])              # [128, ngrp*64]
        r = r.reshape(P, ngrp, D, H).transpose(1, 0, 3, 2)  # -> [g, p, h, d]
        arr = r.reshape(-1, HD)
        out[c * npc + plan["cores"][c]["perm"]] = \
            arr[plan["ndum"]:plan["ndum"] + npc]
    return out, res, plan, in_maps, nc


def kernel(**inputs):
    out, _, _, _, _ = run(**inputs)
    return out


# revision 29
# speedup vs baseline: 1.1753x; 1.0597x over previous
"""Trainium2 Bass kernel for nn_DenTargetTransformerConv (GNN message passing).

Strategy (graph/data parallel, dst-owner sharding across 8 NeuronCores):
  - Nodes are partitioned by dst-id range; each core owns N/8 nodes and all
    edges whose dst falls in its range. The "halo exchange" of src features is
    materialized host-side as per-core edge-slot tables (rows replicated per
    consumer), so the device reads are plain strided DMAs.
  - Per core, own nodes are sorted by in-degree and packed into groups of 128
    (SBUF partition dim). Consecutive groups are merged into equal-K runs
    (K = slots per node, shared across the 8 cores so one program serves all).
  - Edge phase per run: one static DMA brings the [128, RK*128] bf16 q||v
    slot tile; DVE computes per-slot q*k products and exp-weighted v in bf16
    (2x mode); the segment reductions (score over D, aggregation over K) run
    on the Tensor engine as identity-weight PSUM-accumulate matmul chains
    (with one DVE pairwise pre-add stage in front to halve the chain length
    where that balances the engines), and the softmax pad-mask is folded in
    as one extra accumulated matmul of a -400 bias table. exp runs on the
    Scalar engine straight out of PSUM and writes its results interleaved
    into the weighted-v tile, so the softmax denominators fall out of the
    aggregation matmuls as 4 extra PSUM columns per group.
  - v (and everything downstream of the aggregation) lives in a (d,h)
    interleaved layout so the exp broadcast has a step-1 inner axis (DVE 2x
    mode); the host un-permutes the final output.
  - The gate's skip-side logit is a linear function of feat, so it is folded
    into the per-node linears as one extra matmul column. The node phase
    (softmax normalize, gate, LayerNorm, PReLU) runs in three group-chunks
    interleaved with the edge runs; broadcast multiplies go to GpSimd,
    transcendentals to the Scalar engine.
  - Emission is software-pipelined: run i's score phase is emitted before
    run i-1's weighted-aggregation phase, so the in-order DVE stream never
    stalls waiting for a PE/ACT round trip.
"""

import numpy as np
import ml_dtypes

import concourse.bacc as bacc
import concourse.bass as bass
import concourse.tile as tile
from concourse import mybir
from concourse.bass_utils import run_bass_kernel_spmd

F32 = mybir.dt.float32
BF16 = mybir.dt.bfloat16
AX = mybir.AxisListType
ALU = mybir.AluOpType
ACTF = mybir.ActivationFunctionType
BF = ml_dtypes.bfloat16

P = 128
NCORES = 8
HD = 64          # H * D
H, D = 4, 16
IN_F = 64
W68 = HD + H     # weighted-v row + denominator columns
KS = 2 * HD + 2  # per-group ks row: k(64) | skip(64) | lgt | pad

RMAX = 7         # max groups per run (agg PSUM: R*68 <= 476)
RKMAX = 96       # max slot-columns per run
KSPREAD = 3     # max K padding when merging groups into a run
NCHUNK = 5       # node-phase chunks
AGGPRE_R = 2     # agg pairwise pre-add for runs with R <= this

# natural hd = h*16+d  <->  stored j = d*4+h
_PERM = np.arange(HD).reshape(H, D).T.reshape(-1)       # j -> natural hd


# ----------------------------------------------------------------- host prep

def _plan(q_src, v_src, feat, src, dst, ncores):
    n = feat.shape[0]
    npc = n // ncores
    ngrp = (npc + P - 1) // P
    grid = ngrp * P
    ndum = grid - npc

    q2 = np.asarray(q_src, np.float32).reshape(n, HD)
    v2 = np.asarray(v_src, np.float32).reshape(n, H, D).transpose(0, 2, 1).reshape(n, HD)
    qv = np.concatenate([q2, v2], axis=1).astype(BF)    # [n, 128]

    src = np.asarray(src).astype(np.int64)
    dst = np.asarray(dst).astype(np.int64)
    order = np.argsort(dst, kind="stable")
    dst_s, src_s = dst[order], src[order]
    bounds = np.searchsorted(dst_s, np.arange(ncores + 1) * npc)

    cores = []
    gmax = np.zeros((ncores, ngrp), np.int64)
    for c in range(ncores):
        lo, hi = bounds[c], bounds[c + 1]
        dstL = dst_s[lo:hi] - c * npc          # ascending
        srcL = src_s[lo:hi]
        deg = np.bincount(dstL, minlength=npc)
        starts = np.concatenate([[0], np.cumsum(deg)])
        rank = np.arange(len(dstL)) - starts[dstL]
        perm = np.argsort(deg, kind="stable")  # ascending degree
        pos_of = np.empty(npc, np.int64)
        pos_of[perm] = ndum + np.arange(npc)
        gd = np.zeros(grid, np.int64)
        gd[ndum:] = deg[perm]
        gmax[c] = gd.reshape(ngrp, P).max(1)
        cores.append(dict(dstL=dstL, srcL=srcL, rank=rank, perm=perm,
                          pos_of=pos_of))

    K = np.maximum(gmax.max(0), 1)             # per-group slot count

    # merge consecutive groups into equal-K runs (pad K up to the run max)
    runs = []
    g = 0
    while g < ngrp:
        ge = g + 1
        while (ge < ngrp and ge - g < RMAX
               and (ge - g + 1) * K[ge] <= RKMAX
               and K[ge] - K[g] <= KSPREAD):
            ge += 1
        runs.append((g, ge, int(K[ge - 1])))
        g = ge
    rkbase = np.zeros(len(runs) + 1, np.int64)
    for i, (g0, g1, k) in enumerate(runs):
        rkbase[i + 1] = rkbase[i] + (g1 - g0) * k
    totrk = int(rkbase[-1])

    # per-core tables
    per_core = []
    grp_run = np.zeros(ngrp, np.int64)
    for i, (g0, g1, k) in enumerate(runs):
        grp_run[g0:g1] = i
    run_g0 = np.array([r[0] for r in runs])
    run_k = np.array([r[2] for r in runs])

    for c in range(ncores):
        cd = cores[c]
        pos_e = cd["pos_of"][cd["dstL"]]       # grid position of each edge
        g_e = pos_e // P
        p_e = pos_e % P
        i_e = grp_run[g_e]
        r_e = g_e - run_g0[i_e]
        k_e = run_k[i_e]
        # row = rkbase[i]*128 + p*(R*K) + r*K + rank  (partition-major)
        rk_run = np.array([r[1] - r[0] for r in runs])[i_e] * k_e
        rows = rkbase[i_e] * P + p_e * rk_run + r_e * k_e + cd["rank"]
        tabq = np.zeros((totrk * P, HD), BF)
        tabv = np.zeros((totrk * P, HD), BF)
        tabq[rows] = qv[cd["srcL"], :HD]
        tabv[rows] = qv[cd["srcL"], HD:]
        # padded slots have q=v=0 -> score 0 -> exp 1; count them per node
        # so the denominator can be corrected (eps folded in)
        nslot = np.zeros(ngrp, np.int64)
        for i, (g0, g1, k) in enumerate(runs):
            nslot[g0:g1] = k
        real = np.zeros((P, ngrp), np.float32)
        np.add.at(real, (p_e, g_e), 1.0)
        padc = nslot[None, :] - real - 1e-9
        padcnt = np.repeat(padc, H, axis=1).astype(np.float32)  # [128, G*4]
        per_core.append(dict(tabq=tabq, tabv=tabv, padcnt=padcnt))

    # featT with ones row, per core, grid-permuted: [IN_F+1, grid] bf16
    featTs = []
    feat = np.asarray(feat, np.float32)
    for c in range(ncores):
        ft = np.zeros((IN_F + 1, grid), np.float32)
        ft[IN_F, :] = 1.0
        perm = cores[c]["perm"]
        ft[:IN_F, ndum:] = feat[c * npc + perm].T
        featTs.append(ft.astype(BF))

    ident = np.eye(P, dtype=BF)

    return dict(n=n, npc=npc, ngrp=ngrp, grid=grid, ndum=ndum, K=K,
                runs=runs, rkbase=rkbase, totrk=totrk, ident=ident,
                cores=cores, per_core=per_core, featTs=featTs)


# ------------------------------------------------------------- device build

def _build_nc(plan, ncores):
    ngrp, runs, rkbase, totrk = (plan["ngrp"], plan["runs"], plan["rkbase"],
                                 plan["totrk"])
    grid = plan["grid"]
    G = ngrp
    nruns = len(runs)

    nc = bacc.Bacc("TRN2", target_bir_lowering=False, debug=False,
                   num_devices=ncores)

    featT_d = nc.dram_tensor("featT", [IN_F + 1, grid], BF16,
                             kind="ExternalInput").ap()
    FT_G = [12, 12, 12, G - 36]  # featT split sizes (groups)
    tabq_d = nc.dram_tensor("tabq", [totrk * P, HD], BF16,
                            kind="ExternalInput").ap()
    tabv_d = nc.dram_tensor("tabv", [totrk * P, HD], BF16,
                            kind="ExternalInput").ap()
    padc_d = nc.dram_tensor("padcnt", [P, G * H], F32,
                            kind="ExternalInput").ap()
    ident_d = nc.dram_tensor("ident", [P, P], BF16, kind="ExternalInput").ap()
    wcat_d = nc.dram_tensor("wcat", [IN_F + 1, KS], BF16,
                            kind="ExternalInput").ap()
    # bf16 params: [wg2' | gamma' | beta'] (all (d,h)-permuted)
    parb_d = nc.dram_tensor("parb", [1, 3 * HD], BF16,
                            kind="ExternalInput").ap()
    # f32 params: [bgate, prelu_a, eps, pad]
    parf_d = nc.dram_tensor("parf", [1, 4], F32, kind="ExternalInput").ap()
    out_d = nc.dram_tensor("out", [P, G * HD], F32, kind="ExternalOutput").ap()

    # node-phase chunk boundaries; chunk j is emitted in the pipeline slot
    # right after run chunk_done[j]'s aggregation phase
    cw = [12, 12, 11, 9, 5]          # chunk sizes, small tail
    assert sum(cw) == G and len(cw) == NCHUNK
    cb = [sum(cw[:i]) for i in range(NCHUNK + 1)]
    chunk_done = {}
    for j in range(NCHUNK):
        i = min(i for i, (g0, g1, k) in enumerate(runs) if g1 >= cb[j + 1])
        chunk_done.setdefault(i, []).append((cb[j], cb[j + 1]))
    NGMAX = max(cb[j + 1] - cb[j] for j in range(NCHUNK))

    with tile.TileContext(nc) as tc:
        with (
            tc.tile_pool(name="singles", bufs=1) as singles,
            tc.tile_pool(name="plin", bufs=2, space="PSUM") as plin,
            tc.tile_pool(name="pscore", bufs=2, space="PSUM") as pscore,
            tc.tile_pool(name="pagg", bufs=2, space="PSUM") as pagg,
            tc.tile_pool(name="qvp", bufs=3) as qvp,
            tc.tile_pool(name="vvp", bufs=2) as vvp,
            tc.tile_pool(name="prodp", bufs=2) as prodp,
            tc.tile_pool(name="halfp", bufs=2) as halfp,
            tc.tile_pool(name="wp", bufs=2) as wp,
            tc.tile_pool(name="whp", bufs=2) as whp,
            tc.tile_pool(name="nodep", bufs=2) as nodep,
            tc.tile_pool(name="smallp", bufs=2) as smallp,
        ):
            qv_pre = {}
            vv_pre = {}

            def qv_fetch(i):
                g0, g1, K = runs[i]
                RK = (g1 - g0) * K
                r0 = int(rkbase[i])
                qt = qvp.tile([P, RKMAX * HD], BF16, tag="qt")
                in_ap = tabq_d[r0 * P:(r0 + RK) * P, :].rearrange(
                    "(p rk) e -> p (rk e)", p=P)
                nc.sync.dma_start(out=qt[:, :RK * HD], in_=in_ap)
                qv_pre[i] = qt

            def vv_fetch(i):
                g0, g1, K = runs[i]
                RK = (g1 - g0) * K
                r0 = int(rkbase[i])
                vt = vvp.tile([P, RKMAX * HD], BF16, tag="vt")
                in_ap = tabv_d[r0 * P:(r0 + RK) * P, :].rearrange(
                    "(p rk) e -> p (rk e)", p=P)
                nc.sync.dma_start(out=vt[:, :RK * HD], in_=in_ap)
                vv_pre[i] = vt

            # ---- static loads (featT split so linears start early);
            # the first featT quarter and wcat go ahead of the big edge-table
            # prefetches so the linear chain's dependencies land first
            featTs = []
            fg0 = 0
            for j, ng in enumerate(FT_G):
                t = singles.tile([IN_F + 1, ng * P], BF16, tag=f"ft{fg0}")
                nc.sync.dma_start(
                    out=t[:], in_=featT_d[:, fg0 * P:(fg0 + ng) * P])
                featTs.append((fg0, fg0 + ng, t))
                fg0 += ng
                if j == 0:
                    wcat = singles.tile([IN_F + 1, KS], BF16)
                    nc.sync.dma_start(out=wcat[:], in_=wcat_d[:])
                    qv_fetch(0)
                elif j == 1:
                    qv_fetch(1)

            def feat_slice(g):
                for a, b, t in featTs:
                    if a <= g < b:
                        return t[:, (g - a) * P:(g - a + 1) * P]
                raise AssertionError
            ident = singles.tile([P, P], BF16)
            nc.sync.dma_start(out=ident[:], in_=ident_d[:])
            padcnt = singles.tile([P, G * H], F32)
            nc.sync.dma_start(out=padcnt[:], in_=padc_d[:])
            parb = singles.tile([P, 3 * HD], BF16)
            nc.sync.dma_start(
                out=parb[:],
                in_=bass.AP(tensor=parb_d.tensor, offset=parb_d.offset,
                            ap=[[0, P], [1, 3 * HD]]))
            parf = singles.tile([P, 4], F32)
            nc.sync.dma_start(
                out=parf[:],
                in_=bass.AP(tensor=parf_d.tensor, offset=parf_d.offset,
                            ap=[[0, P], [1, 4]]))
            bg_ap = parf[:, 0:1]
            nbg_ap = parf[:, 3:4]
            pa_ap = parf[:, 1:2]
            eps_ap = parf[:, 2:3]

            # persistent state
            ks_bf = singles.tile([P, G * KS], BF16)   # k | skip | lgt | pad
            den = singles.tile([P, G * H], F32)
            agg_bf = singles.tile([P, G * HD], BF16)

            def pap(t, extra, off=0):
                sl = t[:, 0:1]
                return bass.AP(tensor=sl.tensor, offset=sl.offset + off,
                               ap=[sl.ap[0]] + extra)

            # ---- per-node linears: k|skip|lgt = featT_g.T @ wcat
            for c0 in range(0, G, 3):
                cn = min(3, G - c0)
                pl = plin.tile([P, 3 * KS], F32, tag="lin")
                for j in range(cn):
                    nc.tensor.matmul(out=pl[:, j * KS:(j + 1) * KS],
                                     lhsT=feat_slice(c0 + j),
                                     rhs=wcat[:], start=True, stop=True)
                nc.scalar.activation(out=ks_bf[:, c0 * KS:(c0 + cn) * KS],
                                     in_=pl[:, :cn * KS], func=ACTF.Copy)

            # ---- edge phase (software-pipelined emission)
            state = {}

            def score_phase(i):
                g0, g1, K = runs[i]
                R = g1 - g0
                RK = R * K
                r0 = int(rkbase[i])
                if i not in qv_pre:
                    qv_fetch(i)
                qt = qv_pre.pop(i)
                vv_fetch(i)

                # prod[rk, h, d] = q[rk, h, d] * k_g[h, d]  (bf16 2x)
                prod = prodp.tile([P, RKMAX * HD], BF16, tag="prod")
                q3 = pap(qt, [[HD * K, R], [HD, K], [1, HD]])
                kb = pap(ks_bf, [[KS, R], [0, K], [1, HD]], off=g0 * KS)
                p3 = pap(prod, [[HD * K, R], [HD, K], [1, HD]])
                nc.vector.tensor_tensor(out=p3, in0=q3, in1=kb, op=ALU.mult)

                # two pairwise pre-adds: 16 d-slices -> 4
                ph = halfp.tile([P, RKMAX * 32], BF16, tag="ph")
                nc.vector.tensor_tensor(
                    out=pap(ph, [[32, RK], [8, H], [1, 8]]),
                    in0=pap(prod, [[HD, RK], [D, H], [1, 8]]),
                    in1=pap(prod, [[HD, RK], [D, H], [1, 8]], off=8),
                    op=ALU.add)
                pq = halfp.tile([P, RKMAX * 16], BF16, tag="pq")
                nc.vector.tensor_tensor(
                    out=pap(pq, [[16, RK], [4, H], [1, 4]]),
                    in0=pap(ph, [[32, RK], [8, H], [1, 4]]),
                    in1=pap(ph, [[32, RK], [8, H], [1, 4]], off=4),
                    op=ALU.add)

                # score[rk, h] = sum_d4 pq  (PE accumulate)
                sp = pscore.tile([P, RKMAX * H], F32, tag="sp")
                for d in range(4):
                    rhs = pap(pq, [[16, RK], [4, H]], off=d)
                    nc.tensor.matmul(out=sp[:, :RK * H], lhsT=ident[:],
                                     rhs=rhs, start=(d == 0), stop=(d == 3))

                # w68 cols 64:68 = ex = exp(score/4)  (ACT, PSUM -> SBUF)
                w = wp.tile([P, RKMAX * W68], BF16, tag="w")
                nc.scalar.activation(
                    out=pap(w, [[W68, RK], [1, H]], off=HD),
                    in_=sp[:, :RK * H], func=ACTF.Exp, scale=0.25)
                state[i] = w

            def agg_phase(i):
                g0, g1, K = runs[i]
                R = g1 - g0
                RK = R * K
                w = state.pop(i)
                vt = vv_pre.pop(i)

                # w68[rk, 0:64] = v[rk, d, h] * ex[rk, h]  (bf16 2x)
                v3 = pap(vt, [[HD, RK], [H, D], [1, H]])
                eb = pap(w, [[W68, RK], [0, D], [1, H]], off=HD)
                w3 = pap(w, [[W68, RK], [H, D], [1, H]])
                nc.vector.tensor_tensor(out=w3, in0=v3, in1=eb, op=ALU.mult)

                # optional pairwise k pre-add for narrow runs
                ag = pagg.tile([P, RMAX * W68], F32, tag="agg")
                if R <= AGGPRE_R and K > 1:
                    KH = K // 2
                    wh = whp.tile([P, (RKMAX // 2 + 1) * W68], BF16, tag="wh")
                    nc.vector.tensor_tensor(
                        out=pap(wh, [[KH * W68, R], [W68, KH], [1, W68]]),
                        in0=pap(w, [[K * W68, R], [2 * W68, KH], [1, W68]]),
                        in1=pap(w, [[K * W68, R], [2 * W68, KH], [1, W68]],
                                off=W68),
                        op=ALU.add)
                    nmm = KH + (K % 2)
                    for k in range(KH):
                        rhs = pap(wh, [[KH * W68, R], [1, W68]], off=k * W68)
                        nc.tensor.matmul(out=ag[:, :R * W68], lhsT=ident[:],
                                         rhs=rhs, start=(k == 0),
                                         stop=(k == nmm - 1))
                    if K % 2:
                        rhs = pap(w, [[K * W68, R], [1, W68]],
                                  off=(K - 1) * W68)
                        nc.tensor.matmul(out=ag[:, :R * W68], lhsT=ident[:],
                                         rhs=rhs, start=(KH == 0), stop=True)
                else:
                    for k in range(K):
                        rhs = pap(w, [[K * W68, R], [1, W68]], off=k * W68)
                        nc.tensor.matmul(out=ag[:, :R * W68], lhsT=ident[:],
                                         rhs=rhs, start=(k == 0),
                                         stop=(k == K - 1))

                nc.scalar.activation(
                    out=pap(agg_bf, [[HD, R], [1, HD]], off=g0 * HD),
                    in_=pap(ag, [[W68, R], [1, HD]]),
                    func=ACTF.Copy)
                nc.scalar.activation(
                    out=pap(den, [[H, R], [1, H]], off=g0 * H),
                    in_=pap(ag, [[W68, R], [1, H]], off=HD),
                    func=ACTF.Copy)

            nstate = {}

            def node_chunk_a(h0, h1):
                NG = h1 - h0
                Fh = NG * HD
                bce = nc.vector if h1 == G else nc.gpsimd

                dv = smallp.tile([P, NGMAX * H], F32, tag="dinv")
                nc.vector.tensor_tensor(out=dv[:, :NG * H],
                                        in0=den[:, h0 * H:h1 * H],
                                        in1=padcnt[:, h0 * H:h1 * H],
                                        op=ALU.subtract)
                nc.vector.reciprocal_approx_fast(out=dv[:, :NG * H],
                                                 in_=dv[:, :NG * H])
                dvb = smallp.tile([P, NGMAX * H], BF16, tag="dinvb")
                nc.vector.tensor_copy(out=dvb[:, :NG * H],
                                      in_=dv[:, :NG * H])

                # rst = agg * dinv
                rst = nodep.tile([P, NGMAX * HD], BF16, tag="rst")
                dib = pap(dvb, [[H, NG], [0, D], [1, H]])
                a3 = pap(agg_bf, [[HD, NG], [H, D], [1, H]], off=h0 * HD)
                r3 = pap(rst, [[HD, NG], [H, D], [1, H]])
                nc.vector.tensor_tensor(out=r3, in0=a3, in1=dib, op=ALU.mult)

                # gate logit: lgt_skip (from linears) + sum_hd rst*wg2
                zc = nodep.tile([P, NGMAX * HD], BF16, tag="zc")
                sk = pap(ks_bf, [[KS, NG], [1, HD]], off=h0 * KS + HD)
                wg2 = pap(parb, [[0, NG], [1, HD]], off=0)
                nc.vector.tensor_tensor(out=zc[:, :Fh], in0=rst[:, :Fh],
                                        in1=wg2, op=ALU.mult)
                lgs = smallp.tile([P, NGMAX], F32, tag="lgs")
                nc.vector.tensor_reduce(
                    out=lgs[:, :NG],
                    in_=pap(zc, [[HD, NG], [1, HD]]),
                    axis=AX.X, op=ALU.add)
                logit = smallp.tile([P, NGMAX], F32, tag="logit")
                lgtb = pap(ks_bf, [[KS, NG], [1, 1]], off=h0 * KS + 2 * HD)
                nc.vector.tensor_tensor(out=logit[:, :NG], in0=lgs[:, :NG],
                                        in1=lgtb, op=ALU.add)
                ge = smallp.tile([P, NGMAX], F32, tag="ge")
                nc.scalar.activation(out=ge[:, :NG], in_=logit[:, :NG],
                                     func=ACTF.Exp, scale=-1.0, bias=nbg_ap)
                nc.vector.tensor_scalar(out=ge[:, :NG], in0=ge[:, :NG],
                                        scalar1=1.0, scalar2=None,
                                        op0=ALU.add)
                nc.vector.reciprocal_approx_fast(out=ge[:, :NG],
                                                 in_=ge[:, :NG])
                gate = smallp.tile([P, NGMAX], BF16, tag="gate")
                nc.vector.tensor_copy(out=gate[:, :NG], in_=ge[:, :NG])

                # rst += gate * (skip - rst)
                dif = nodep.tile([P, NGMAX * HD], BF16, tag="dif")
                nc.vector.tensor_tensor(out=dif[:, :Fh], in0=sk,
                                        in1=rst[:, :Fh], op=ALU.subtract)
                gb = pap(gate, [[1, NG], [0, HD]])
                d3 = pap(dif, [[HD, NG], [1, HD]])
                bce.tensor_tensor(out=d3, in0=d3, in1=gb, op=ALU.mult)
                nstate[h0] = (h1, rst, dif, bce)

            def node_chunk_bc(h0):
                h1, rst, dif, bce = nstate.pop(h0)
                NG = h1 - h0
                Fh = NG * HD
                nc.vector.tensor_tensor(out=rst[:, :Fh], in0=rst[:, :Fh],
                                        in1=dif[:, :Fh], op=ALU.add)

                # LayerNorm stats: sum rst (DVE), sum rst^2 (ACT square)
                sq = nodep.tile([P, NGMAX * HD], BF16, tag="zc2")
                nc.scalar.activation(out=sq[:, :Fh], in_=rst[:, :Fh],
                                     func=ACTF.Square)
                stats = smallp.tile([P, 2 * NGMAX], F32, tag="stats")
                nc.vector.tensor_reduce(
                    out=stats[:, :NG],
                    in_=pap(rst, [[HD, NG], [1, HD]]),
                    axis=AX.X, op=ALU.add)
                nc.vector.tensor_reduce(
                    out=stats[:, NG:2 * NG],
                    in_=pap(sq, [[HD, NG], [1, HD]]),
                    axis=AX.X, op=ALU.add)
                nc.vector.tensor_scalar(out=stats[:, :2 * NG],
                                        in0=stats[:, :2 * NG],
                                        scalar1=1.0 / HD, scalar2=None,
                                        op0=ALU.mult)
                mu = stats[:, 0:NG]
                msq = stats[:, NG:2 * NG]
                var = smallp.tile([P, NGMAX], F32, tag="var")
                nc.vector.tensor_tensor(out=var[:, :NG], in0=mu, in1=mu,
                                        op=ALU.mult)
                nc.vector.tensor_tensor(out=var[:, :NG], in0=msq,
                                        in1=var[:, :NG], op=ALU.subtract)
                sd = smallp.tile([P, NGMAX], F32, tag="sd")
                nc.scalar.activation(out=sd[:, :NG], in_=var[:, :NG],
                                     func=ACTF.Sqrt, bias=eps_ap)
                nc.vector.reciprocal_approx_fast(out=sd[:, :NG],
                                                 in_=sd[:, :NG])
                mrs = smallp.tile([P, 2 * NGMAX], BF16, tag="mrs")
                nc.vector.tensor_copy(out=mrs[:, :NG], in_=mu)
                nc.vector.tensor_copy(out=mrs[:, NG:2 * NG], in_=sd[:, :NG])

                # xhat = (rst - mu) * rstd; out = prelu(xhat*gamma + beta)
                mub = pap(mrs, [[1, NG], [0, HD]])
                bce.tensor_tensor(out=rst[:, :Fh], in0=rst[:, :Fh],
                                  in1=mub, op=ALU.subtract)
                rsb = pap(mrs, [[1, NG], [0, HD]], off=NG)
                bce.tensor_tensor(out=rst[:, :Fh], in0=rst[:, :Fh],
                                  in1=rsb, op=ALU.mult)
                gmb = pap(parb, [[0, NG], [1, HD]], off=HD)
                nc.vector.tensor_tensor(out=rst[:, :Fh], in0=rst[:, :Fh],
                                        in1=gmb, op=ALU.mult)
                btb = pap(parb, [[0, NG], [1, HD]], off=2 * HD)
                nc.vector.tensor_tensor(out=rst[:, :Fh], in0=rst[:, :Fh],
                                        in1=btb, op=ALU.add)
                outf = nodep.tile([P, NGMAX * HD], F32, tag="outf")
                nc.scalar.activation(out=outf[:, :Fh], in_=rst[:, :Fh],
                                     func=ACTF.Prelu, alpha=pa_ap)
                nc.sync.dma_start(out=out_d[:, h0 * HD:h1 * HD],
                                  in_=outf[:, :Fh])

            pend = []
            for i in range(nruns + 1):
                if i < nruns:
                    score_phase(i)
                for h0 in pend:
                    node_chunk_bc(h0)
                pend = []
                if i > 0:
                    agg_phase(i - 1)
                    for h0, h1 in chunk_done.get(i - 1, []):
                        node_chunk_a(h0, h1)
                        pend.append(h0)
            for h0 in pend:
                node_chunk_bc(h0)

    nc.compile()
    return nc


# ------------------------------------------------------------------- driver

_CACHE = {}


def _get_nc(plan, ncores):
    key = (tuple(int(k) for g0, g1, k in plan["runs"]),
           tuple(g1 - g0 for g0, g1, k in plan["runs"]),
           plan["grid"], ncores)
    if key not in _CACHE:
        _CACHE[key] = _build_nc(plan, ncores)
    return _CACHE[key]


def _make_inmaps(plan, params, ncores):
    (Wk, bk, Wskip, bskip, Wgate, bgate, ln_gamma, ln_beta, prelu_a) = params
    Wk = np.asarray(Wk, np.float32)
    bk = np.asarray(bk, np.float32)
    Wsk = np.asarray(Wskip, np.float32)
    bsk = np.asarray(bskip, np.float32)
    wg = np.asarray(Wgate, np.float32).reshape(3 * HD)
    wg1n = wg[0:HD] + wg[2 * HD:3 * HD]                 # acts on skip (nat)
    wcat = np.zeros((IN_F + 1, KS), np.float32)
    wcat[:IN_F, :HD] = Wk
    wcat[IN_F, :HD] = bk
    wcat[:IN_F, HD:2 * HD] = Wsk[:, _PERM]
    wcat[IN_F, HD:2 * HD] = bsk[_PERM]
    wcat[:IN_F, 2 * HD] = Wsk @ wg1n                    # skip-side gate logit
    wcat[IN_F, 2 * HD] = bsk @ wg1n
    wcat = wcat.astype(BF)

    parb = np.zeros((1, 3 * HD), np.float32)
    parb[0, 0:HD] = (wg[HD:2 * HD] - wg[2 * HD:3 * HD])[_PERM]   # on rst
    parb[0, HD:2 * HD] = np.asarray(ln_gamma, np.float32)[_PERM]
    parb[0, 2 * HD:3 * HD] = np.asarray(ln_beta, np.float32)[_PERM]
    parb = parb.astype(BF)
    parf = np.zeros((1, 4), np.float32)
    parf[0, 0] = np.float32(np.asarray(bgate).reshape(-1)[0])
    parf[0, 1] = np.float32(np.asarray(prelu_a).reshape(-1)[0])
    parf[0, 2] = 1e-5
    parf[0, 3] = -parf[0, 0]

    in_maps = []
    for c in range(ncores):
        pc = plan["per_core"][c]
        m = dict(featT=plan["featTs"][c], tabq=pc["tabq"],
                 tabv=pc["tabv"], padcnt=pc["padcnt"], ident=plan["ident"],
                 wcat=wcat, parb=parb, parf=parf)
        in_maps.append(m)
    return in_maps


def run(q_src, v_src, feat, src, dst, Wk, bk, Wskip, bskip, Wgate, bgate,
        ln_gamma, ln_beta, prelu_a, ncores=NCORES, trace=False):
    plan = _plan(q_src, v_src, feat, src, dst, ncores)
    nc = _get_nc(plan, ncores)
    in_maps = _make_inmaps(
        plan, (Wk, bk, Wskip, bskip, Wgate, bgate, ln_gamma, ln_beta, prelu_a),
        ncores)
    res = run_bass_kernel_spmd(nc, in_maps, core_ids=list(range(ncores)),
                               trace=trace)
    n, npc, ngrp = plan["n"], plan["npc"], plan["ngrp"]
    out = np.empty((n, HD), np.float32)
    for c in range(ncores):
        r = np.asarray(res.results[c]["out"])              # [128, ngrp*64]
        r = r.reshape(P, ngrp, D, H).transpose(1, 0, 3, 2)  # -> [g, p, h, d]
        arr = r.reshape(-1, HD)
        out[c * npc + plan["cores"][c]["perm"]] = \
            arr[plan["ndum"]:plan["ndum"] + npc]
    return out, res, plan, in_maps, nc


def kernel(**inputs):
    out, _, _, _, _ = run(**inputs)
    return out
